# revision 24
# baseline (speedup 1.0000x reference)
"""Multi-head attention (AnyAttention) on 8 TRN2 NeuronCores.

Sharding: (batch, head-group): core i handles batch i//4 and heads
4*(i%4) .. 4*(i%4)+4 over ALL 2048 queries (tensor parallel on heads,
row-parallel output projection).  Each core emits a bf16 partial
output [2048, 1024] = attn_out_mine @ Wp[mine_rows]; the host sums
the 4 partials per batch in fp32.

Per-core pipeline (v2 schedule):
  - qT/kT computed c-major [128(=2 heads x 64), 2048]; v token-major
    [128 tok, 4 heads, 65] (65th col = ones -> softmax denominator
    rides the PV matmul stream)
  - logits S^T[k, q]: per (head-pair, qch=512, kt): both heads' QK
    matmuls at PE row bases 0/64 into the two banks of one
    [128, 1024] psum tile; ONE [128, 1024] exp covers both heads
  - mask applied post-exp as mul by host-prepped (1-mask)^T on
    DVE (bf16 2x mode); a few early-slot muls go to GpSimd
  - PV trails QK/exp by LAG=2 slots; the 2 leftover pairs of each
    block are CARRIED into the next block's first two slots so the
    ACT exp cadence never pauses at a block boundary
  - PV psum is double-buffered by block parity (pvA0/pvA1 vs
    pvB0/pvB1, one bank each) so the next block's PV start never
    WAR-waits on the previous block's normalize chain
  - softmax 1/denom: per head, copy the denominator row to
    partition 0, reciprocal_approx_fast [1,512], then
    gpsimd.partition_broadcast into a [64,512] bcs block; the
    normalize multiply reads PV straight out of PSUM.  No PE
    broadcast matmul and no PSUM util bank needed for finalize.
  - proj / outproj / vproj psum lives in the OPPOSITE parity's pv
    banks (free window: after that parity's finalize, before its
    next block), or the CURRENT parity's banks at pk0-1 (free until
    pv_pair(0) at pk2)
  - DMA: critical prefix wq,wk,xt (sync+scalar HW queues) before
    wv, mask0, wp so the first QK can start ~15us in
  - all matmuls bf16 with fp32 PSUM accumulation; scale 1/sqrt(c)
    folded into Wq on host; bp added on host
"""

import numpy as np
import ml_dtypes

B, N, D = 2, 2048, 1024
G, C = 16, 64          # heads, head dim
HPC = 4                # heads per core
NCORES = 8
NQCH = 4               # query chunks of 512
QCH = N // NQCH
TT = N // 128          # 16 token/key tiles
KT = D // 128          # 8 contraction tiles over d

BF16 = ml_dtypes.bfloat16

_cache = {}


def _import_concourse():
    try:
        import concourse.bass  # noqa: F401
    except ImportError:
        import sys
        sys.path.insert(0, "/opt/trn_rl_repo")


def _build():
    _import_concourse()
    import concourse.bass as bass  # noqa: F401
    from concourse import bacc, mybir
    import concourse.tile as tile

    fp32 = mybir.dt.float32
    bf16 = mybir.dt.bfloat16
    AF = mybir.ActivationFunctionType

    nc = bacc.Bacc("TRN2", target_bir_lowering=False, debug=False,
                   num_devices=NCORES)

    # ---- DRAM I/O (per-core shards; same program on all cores) ----
    xt = nc.dram_tensor("xt", [D, N], bf16, kind="ExternalInput").ap()
    wq = nc.dram_tensor("wq", [D, 256], bf16, kind="ExternalInput").ap()
    wk = nc.dram_tensor("wk", [D, 256], bf16, kind="ExternalInput").ap()
    wv = nc.dram_tensor("wv", [D, 256], bf16, kind="ExternalInput").ap()
    wp = nc.dram_tensor("wp", [256, D], bf16, kind="ExternalInput").ap()
    maskt = nc.dram_tensor("maskt", [NQCH, TT, 128, QCH], bf16,
                           kind="ExternalInput").ap()
    out3 = nc.dram_tensor("out3", [1, 2 * QCH], fp32,
                          kind="ExternalOutput").ap()
    out = nc.dram_tensor("out", [N, D], bf16, kind="ExternalOutput").ap()
    # UNNORMALIZED per-head h1 partials of the LAST qch's output
    # rows + their softmax denominators; the host normalizes and
    # adds.  Keeps the end-of-kernel drain off the whole finalize
    # chain (recip -> broadcast -> normalize).
    out2 = nc.dram_tensor("out2", [2, QCH, D], bf16,
                          kind="ExternalOutput").ap()

    GPS_MULS = 0   # h2=1 mask muls of the LAST GPS_MULS pks go to GpSimd
    # (GpSimd muls run concurrently with DVE muls on the same mask
    # slice and SBUF contention slows the DVE mul 0.6us -> 2.5us;
    # all-DVE is faster overall)

    with tile.TileContext(nc) as tc:
        with (
            tc.tile_pool(name="wts", bufs=3) as wpool,
            tc.tile_pool(name="wpp", bufs=1) as wppool,
            tc.tile_pool(name="xtp", bufs=1) as xtpool,
            tc.tile_pool(name="maskp", bufs=2) as maskpool,
            tc.tile_pool(name="stay", bufs=1) as stay,
            tc.tile_pool(name="etp", bufs=2) as etpool,
            tc.tile_pool(name="metp", bufs=3) as metpool,
            tc.tile_pool(name="bcsp", bufs=2) as bcsp,
            tc.tile_pool(name="small", bufs=1) as small,
            tc.tile_pool(name="outp", bufs=2) as outpool,
            tc.tile_pool(name="psqk", bufs=2, space="PSUM") as psqk,
            tc.tile_pool(name="pspv", bufs=1, space="PSUM") as pspv,
        ):
            # ---------- DMA: critical prefix first ----------
            # sync HW queue: wq, wk, xt evens; scalar HW queue: xt
            # odds, wv, mask0, wp.  The first QK needs wq+wk+xt; wv
            # by ~pk0 of block 0; mask0 by the first mask mul; wp
            # not until the first outproj (block 2).
            w_t = {}
            for name, dr in (("wq", wq), ("wk", wk)):
                wt = wpool.tile([128, KT * 256], bf16, tag=name, name=name)
                nc.sync.dma_start(
                    out=wt.rearrange("p (dk c) -> p dk c", dk=KT),
                    in_=dr.rearrange("(dk p) c -> p dk c", p=128))
                w_t[name] = [wt[:, dk * 256:(dk + 1) * 256]
                             for dk in range(KT)]
            xt_t = []
            for dk in range(KT):
                t = xtpool.tile([128, N], bf16, tag=f"xt{dk}")
                eng = nc.sync if dk % 2 == 0 else nc.scalar
                eng.dma_start(out=t, in_=xt[dk * 128:(dk + 1) * 128, :])
                xt_t.append(t)
            for name, dr in (("wv", wv),):
                wt = wpool.tile([128, KT * 256], bf16, tag=name, name=name)
                nc.scalar.dma_start(
                    out=wt.rearrange("p (dk c) -> p dk c", dk=KT),
                    in_=dr.rearrange("(dk p) c -> p dk c", p=128))
                w_t[name] = [wt[:, dk * 256:(dk + 1) * 256]
                             for dk in range(KT)]

            mask_t = {}

            def load_mask(qch, eng=None):
                mt = maskpool.tile([128, TT * QCH], bf16, tag="mask")
                (eng or nc.sync).dma_start(
                    out=mt.rearrange("p (kt q) -> p kt q", kt=TT),
                    in_=maskt[qch].rearrange("kt p q -> p kt q"))
                mask_t[qch] = mt

            load_mask(0, eng=nc.scalar)
            wpt = wppool.tile([128, 2 * D], bf16, tag="wp", name="wp")
            nc.scalar.dma_start(
                out=wpt.rearrange("p (r c) -> p r c", r=2),
                in_=wp.rearrange("(r p) c -> p r c", p=128))
            w_t["wp"] = [wpt[:, r * D:(r + 1) * D] for r in range(2)]
            # head-1 half of Wp's hp1 rows re-based to partition 0 for
            # the per-head unnormalized tail matmuls
            wp1b = stay.tile([C, D], bf16, tag="wp1b", name="wp1b")

            # touch Exp now so ACT_TABLE_LOAD is off the critical path
            warm = small.tile([1, C], bf16, tag="warm")
            nc.vector.memset(warm, 1.0)
            nc.scalar.activation(out=warm, in_=warm, func=AF.Exp)
            nc.vector.tensor_copy(out=wp1b,
                                  in_=wpt[C:2 * C, D:2 * D])

            qT = [stay.tile([128, N], bf16, tag=f"qT{hp}", name=f"qT{hp}")
                  for hp in range(2)]
            kTt = [stay.tile([128, N], bf16, tag=f"kT{hp}", name=f"kT{hp}")
                   for hp in range(2)]
            aT = [stay.tile([128, N], bf16, tag=f"aT{hp}", name=f"aT{hp}")
                  for hp in range(2)]
            v_t = [stay.tile([128, HPC, C + 1], bf16, tag=f"v{tt}",
                             name=f"v{tt}")
                   for tt in range(TT)]

            # ---------- PSUM: 4 banks QK + 4 banks PV (parity) ----------
            # pv tiles double as proj/outproj/vproj scratch ("util")
            # during their idle parity windows.
            def pv_tile(par, h2):
                return pspv.tile([128, QCH], fp32, tag=f"pv{par}{h2}",
                                 name=f"pv{par}{h2}")

            util_state = {"par": 0, "h2": 0}

            def util_tile():
                # rotate over the two banks of the currently-safe parity
                h2 = util_state["h2"]
                util_state["h2"] = 1 - h2
                return pv_tile(util_state["par"], h2)

            # ---------- worker emitters ----------
            def _proj_half(wname, dst, hp, ch, part, box):
                if part == 0:
                    box.clear()
                    box.append(util_tile())
                ps = box[0]
                for dk in range(part * 4, part * 4 + 4):
                    nc.tensor.matmul(
                        ps, w_t[wname][dk][:, hp * 128:(hp + 1) * 128],
                        xt_t[dk][:, ch * QCH:(ch + 1) * QCH],
                        start=(dk == 0), stop=(dk == KT - 1))
                if part == 1:
                    nc.vector.tensor_copy(
                        out=dst[hp][:, ch * QCH:(ch + 1) * QCH], in_=ps)

            def qproj_group(hp, qch, part=None, box=[]):
                if part is None:
                    _proj_half("wq", qT, hp, qch, 0, box)
                    _proj_half("wq", qT, hp, qch, 1, box)
                else:
                    _proj_half("wq", qT, hp, qch, part, box)

            def kproj_group(hp, ch, part=None, box=[]):
                if part is None:
                    _proj_half("wk", kTt, hp, ch, 0, box)
                    _proj_half("wk", kTt, hp, ch, 1, box)
                else:
                    _proj_half("wk", kTt, hp, ch, part, box)

            def vproj_group(tt):
                ps = util_tile()
                for dk in range(KT):
                    nc.tensor.matmul(
                        ps[:, 0:256],
                        xt_t[dk][:, tt * 128:(tt + 1) * 128],
                        w_t["wv"][dk],
                        start=(dk == 0), stop=(dk == KT - 1))
                vt = v_t[tt]
                nc.vector.memset(vt[:, :, C:C + 1], 1.0)
                nc.vector.tensor_copy(
                    out=vt[:, :, 0:C],
                    in_=ps[:, 0:256].rearrange("p (h c) -> p h c", c=C))

            def outproj_h0_slab(tt2):
                # normal h0 output rows for the LAST qch
                r0 = (NQCH - 1) * QCH + tt2 * 128
                osb = outpool.tile([128, D], bf16, tag="osb")
                for dch in range(2):
                    ps = util_tile()
                    nc.tensor.matmul(
                        ps, aT[0][:, r0:r0 + 128],
                        w_t["wp"][0][:, dch * QCH:(dch + 1) * QCH],
                        start=True, stop=True)
                    nc.vector.tensor_copy(
                        out=osb[:, dch * QCH:(dch + 1) * QCH], in_=ps)
                nc.sync.dma_start(out=out[r0:r0 + 128, :], in_=osb)

            def outproj_h1_unnorm(pvs, h2, tt2):
                # per-head UNNORMALIZED h1 output rows of the last
                # qch, straight from the pvs cast (no finalize wait)
                osb = outpool.tile([128, D], bf16, tag="osb")
                wphalf = w_t["wp"][1] if h2 == 0 else None
                for dch in range(2):
                    ps = util_tile()
                    if h2 == 0:
                        rhs = w_t["wp"][1][0:C,
                                           dch * QCH:(dch + 1) * QCH]
                    else:
                        rhs = wp1b[:, dch * QCH:(dch + 1) * QCH]
                    nc.tensor.matmul(
                        ps[:, :],
                        pvs[:, h2 * QCH + tt2 * 128:
                            h2 * QCH + (tt2 + 1) * 128],
                        rhs, start=True, stop=True)
                    nc.vector.tensor_copy(
                        out=osb[:, dch * QCH:(dch + 1) * QCH], in_=ps)
                nc.sync.dma_start(
                    out=out2[h2, tt2 * 128:(tt2 + 1) * 128, :], in_=osb)

            osb_box = {}

            def outproj_half(qch, tt2, dch):
                # one dch half of one 128-token output row block; the
                # DMA fires with the second half
                if dch == 0:
                    osb_box[tt2] = outpool.tile([128, D], bf16,
                                                tag="osb", name="osb")
                osb = osb_box[tt2]
                r0 = qch * QCH + tt2 * 128
                ps = util_tile()
                for hp in range(2):
                    nc.tensor.matmul(
                        ps, aT[hp][:, r0:r0 + 128],
                        w_t["wp"][hp][:, dch * QCH:(dch + 1) * QCH],
                        start=(hp == 0), stop=(hp == 1))
                nc.vector.tensor_copy(
                    out=osb[:, dch * QCH:(dch + 1) * QCH], in_=ps)
                if dch == 1:
                    nc.sync.dma_start(out=out[r0:r0 + 128, :], in_=osb)

            # ---------- attention block for (qch, hp) ----------
            LAG = 2

            def attn_block(blk, qch, hp, interleave, carry_in,
                           last=False):
                par = blk % 2
                mt = mask_t[qch]
                pv = [pv_tile(par, h2)[0:C + 1, :] for h2 in range(2)]
                # et/met tiles cover a 2-pair UNIT (4 kt): the mask
                # multiply is then ONE out-of-place [128,2048] bf16
                # tensor_tensor per head per unit, which the DVE runs
                # in 4x mode (~690ns vs 2x 601ns per [128,1024])
                et_units = [None] * (TT // 4)
                met_units = [None] * (TT // 4)

                def pv_pair(pk2, pv=pv, hp=hp):
                    metu = met_units[pk2 // 2]
                    pr = pk2 % 2
                    for j in range(2):
                        kt = 2 * pk2 + j
                        for h2 in range(2):
                            nc.tensor.matmul(
                                pv[h2], v_t[kt][:, hp * 2 + h2, :],
                                metu[:, h2, pr, j, :],
                                start=(kt == 0), stop=(kt == TT - 1))

                for pk in range(TT // 2):
                    u, pr = pk // 2, pk % 2
                    if pr == 0:
                        et_units[u] = etpool.tile(
                            [128, 2, 2, 2, QCH], bf16, tag="et",
                            name="et")
                        met_units[u] = metpool.tile(
                            [128, 2, 2, 2, QCH], bf16, tag="met",
                            name="met")
                    et = et_units[u]
                    for j in ((0, 1) if pk % 2 == 0 else (1, 0)):
                        kt = 2 * pk + j
                        ps = psqk.tile([128, 2 * QCH], fp32, tag="qk")
                        for h2 in range(2):
                            pb = h2 * C
                            nc.tensor.matmul(
                                ps[:, h2 * QCH:(h2 + 1) * QCH],
                                kTt[hp][pb:pb + C,
                                        kt * 128:(kt + 1) * 128],
                                qT[hp][pb:pb + C,
                                       qch * QCH:(qch + 1) * QCH],
                                start=True, stop=True)
                        nc.scalar.activation(
                            out=et[:, :, pr, j, :],
                            in_=ps.rearrange("p (h q) -> p h q", h=2),
                            func=AF.Exp)
                    # carry first: its PE ops (tail pv pairs, pk0-1)
                    # depend only on the previous block's tiles, and
                    # its DVE ops (finalize stage 1, pk2) must beat
                    # this slot's mask muls into the DVE FIFO so the
                    # pv banks free on time
                    if carry_in is not None and pk < len(carry_in):
                        carry_in[pk]()
                    if pr == 1:
                        met = met_units[u]
                        msl = mt[:, u * 4 * QCH:(u + 1) * 4 * QCH]
                        for h2 in range(2):
                            nc.vector.tensor_mul(
                                met[:, h2].rearrange(
                                    "p a b c -> p (a b c)"),
                                et[:, h2].rearrange(
                                    "p a b c -> p (a b c)"),
                                msl)
                    if pk >= LAG:
                        pv_pair(pk - LAG)
                    for f in interleave.get(pk, ()):
                        f()

                # Finalize, staged across carry slots so no DVE op
                # ever waits in-FIFO on the gpsimd broadcast:
                #   pk2: denominator rows -> [1,1024] row, pv value
                #        rows -> SBUF bf16 (frees the pv psum banks)
                #   pk3: recip [1,1024] + bf16 cast + gpsimd
                #        broadcast to [64,1024]
                #   pk5: 2x-mode bf16 normalize multiplies into aT
                fin_box = {}

                def fin_stage1(pv=pv):
                    # recip chain first (it gates the bc matmuls at
                    # pk3), then the pv value casts (they gate only
                    # the pk4+ norm muls / bank release)
                    dga = small.tile([1, 2 * QCH], fp32, tag="dga")
                    for h2 in range(2):
                        nc.vector.tensor_copy(
                            out=dga[:, h2 * QCH:(h2 + 1) * QCH],
                            in_=pv[h2][C:C + 1, :])
                    rcf = small.tile([1, 2 * QCH], fp32, tag="rcf")
                    nc.vector.reciprocal_approx_fast(rcf, dga)
                    rcb = small.tile([1, 2 * QCH], bf16, tag="rcb")
                    nc.vector.tensor_copy(out=rcb, in_=rcf)
                    fin_box["rcb"] = rcb
                    pvs = bcsp.tile([C, 2 * QCH], bf16, tag="pvs",
                                    name="pvs")
                    for h2 in range(2):
                        nc.vector.tensor_copy(
                            out=pvs[:, h2 * QCH:(h2 + 1) * QCH],
                            in_=pv[h2][0:C, :])
                    fin_box["pvs"] = pvs

                def fin_bc():
                    # broadcast the recip row to 64 partitions on the
                    # (otherwise idle) GpSimd -- its library never
                    # switches, so no reload cost
                    bcs = bcsp.tile([C, 2 * QCH], bf16, tag="bcs",
                                    name="bcs")
                    nc.gpsimd.partition_broadcast(bcs, fin_box["rcb"])
                    fin_box["bcs"] = bcs

                def fin_norm(hp=hp, qch=qch):
                    pvs, bcs = fin_box["pvs"], fin_box["bcs"]
                    for h2 in range(2):
                        nc.vector.tensor_mul(
                            aT[hp][h2 * C:(h2 + 1) * C,
                                   qch * QCH:(qch + 1) * QCH],
                            pvs[:, h2 * QCH:(h2 + 1) * QCH],
                            bcs[:, h2 * QCH:(h2 + 1) * QCH])

                # carry closures: the LAG leftover pv pairs + the
                # staged finalize, woven into the NEXT block's slots
                def tail0():
                    pv_pair(TT // 2 - LAG)

                def tail1():
                    pv_pair(TT // 2 - LAG + 1)

                if last:
                    # no on-chip normalize for the final block: ship
                    # raw denominators + unnormalized pvs
                    def last_fin(pv=pv):
                        dga = small.tile([1, 2 * QCH], fp32, tag="dga")
                        pvs = bcsp.tile([C, 2 * QCH], bf16, tag="pvs",
                                        name="pvs")
                        for h2 in range(2):
                            nc.vector.tensor_copy(
                                out=dga[:, h2 * QCH:(h2 + 1) * QCH],
                                in_=pv[h2][C:C + 1, :])
                            nc.vector.tensor_copy(
                                out=pvs[:, h2 * QCH:(h2 + 1) * QCH],
                                in_=pv[h2][0:C, :])
                        nc.scalar.dma_start(out=out3, in_=dga)
                        fin_box["pvs"] = pvs

                    return [tail0, tail1, last_fin], fin_box
                return [tail0, tail1, fin_stage1, fin_bc, fin_norm]

            # ---------- emission schedule ----------
            # prologue: kproj(0, ch0) + qproj(0, qch0) halves
            # interleaved so both accumulation chains track the
            # per-dk xt DMA arrivals
            util_state["par"] = 1   # block 0 (par 0) borrows pvB
            bq_p, bk_p = [], []
            qproj_group(0, 0, 0, bq_p)
            kproj_group(0, 0, 0, bk_p)
            qproj_group(0, 0, 1, bq_p)
            kproj_group(0, 0, 1, bk_p)

            carry = None
            blk = 0
            for qch in range(NQCH):
                if qch + 1 < NQCH:
                    load_mask(qch + 1,
                              eng=(nc.sync if qch % 2 == 0 else nc.scalar))
                for hp in range(2):
                    par = blk % 2
                    il = {}

                    def add(pk, f):
                        il.setdefault(pk, []).append(f)

                    # util banks: pk0-1 -> current parity (free until
                    # pv_pair(0) at pk2); pk3+ -> opposite parity
                    # (pv value/denom rows are copied out ~1.6us into
                    # the carried finalize, so those banks free ~pk3)
                    def set_util(p):
                        def f(p=p):
                            util_state["par"] = p
                        return f

                    add(0, set_util(par))
                    add(3, set_util(1 - par))

                    if qch == 0 and hp == 0:
                        # v tiles (2p, 2p+1) consumed by pv_pair(p) at
                        # pk=p+2 (tails p=6,7 in the next block's
                        # pk0,1); k chunks ch needed by QK at pk=2ch.
                        # NOTE pk2 is a util-bank hole: own-parity
                        # banks start accumulating PV at pk2, opposite
                        # banks are read by the carried finalize until
                        # ~pk3.
                        add(0, lambda: vproj_group(0))
                        add(0, lambda: vproj_group(1))
                        add(0, lambda: kproj_group(0, 1))
                        add(1, lambda: vproj_group(2))
                        add(1, lambda: vproj_group(3))
                        add(1, lambda: kproj_group(0, 2))
                        add(3, lambda: vproj_group(4))
                        add(3, lambda: vproj_group(5))
                        add(3, lambda: kproj_group(0, 3))
                        add(4, lambda: vproj_group(6))
                        add(4, lambda: vproj_group(7))
                        add(5, lambda: vproj_group(8))
                        add(5, lambda: vproj_group(9))
                        add(5, lambda: kproj_group(1, 0))
                        add(6, lambda: vproj_group(10))
                        add(6, lambda: vproj_group(11))
                        bq1 = []
                        add(6, lambda: qproj_group(1, 0, 0, bq1))
                        add(7, lambda: vproj_group(12))
                        add(7, lambda: vproj_group(13))
                        add(7, lambda: qproj_group(1, 0, 1, bq1))
                    if qch == 0 and hp == 1:
                        b1, b2, b3 = [], [], []
                        add(0, lambda: vproj_group(14))
                        add(0, lambda: vproj_group(15))
                        add(0, lambda: kproj_group(1, 1, 0, b1))
                        add(1, lambda: kproj_group(1, 1, 1, b1))
                        add(3, lambda: kproj_group(1, 2, 0, b2))
                        add(4, lambda: kproj_group(1, 2, 1, b2))
                        add(5, lambda: kproj_group(1, 3, 0, b3))
                        add(6, lambda: kproj_group(1, 3, 1, b3))
                    if hp == 0 and qch > 0:
                        # qproj for this qch's hp1 block (needed by its
                        # pk0); opposite banks free ~pk3
                        bq = []
                        add(3, lambda q=qch: qproj_group(1, q, 0, bq))
                        add(4, lambda q=qch: qproj_group(1, q, 1, bq))
                        # output projection of the previous qch, in
                        # dch-halves (~0.45us each) so no single slot
                        # overruns; aT ready ~pk4.5
                        add(5, lambda q=qch - 1: outproj_half(q, 0, 0))
                        add(6, lambda q=qch - 1: outproj_half(q, 0, 1))
                        add(7, lambda q=qch - 1: outproj_half(q, 1, 0))
                    if hp == 1 and qch > 0:
                        add(3, lambda q=qch - 1: outproj_half(q, 1, 1))
                        add(4, lambda q=qch - 1: outproj_half(q, 2, 0))
                        add(5, lambda q=qch - 1: outproj_half(q, 2, 1))
                        add(6, lambda q=qch - 1: outproj_half(q, 3, 0))
                        add(7, lambda q=qch - 1: outproj_half(q, 3, 1))
                    if hp == 1 and qch + 1 < NQCH:
                        bq0 = []
                        add(4, lambda q=qch: qproj_group(0, q + 1, 0, bq0))
                        add(5, lambda q=qch: qproj_group(0, q + 1, 1, bq0))
                    if hp == 1 and qch == NQCH - 1:
                        # aT[0][q3] lands ~pk4.5 (finalize of (q3,h0)
                        # carried into this block)
                        add(5, lambda: outproj_h0_slab(0))
                        add(6, lambda: outproj_h0_slab(1))
                        add(7, lambda: outproj_h0_slab(2))
                    last = (qch == NQCH - 1 and hp == 1)
                    r = attn_block(blk, qch, hp, il, carry, last=last)
                    if last:
                        carry, last_box = r
                    else:
                        carry = r
                    blk += 1
            # last block: tails + raw-denominator export, then the
            # unnormalized per-head h1 output rows (no finalize wait)
            for f in carry:
                f()
            util_state["par"] = 0   # last block was par 1
            outproj_h0_slab(3)
            pvs_last = last_box["pvs"]
            for h2 in range(2):
                for tt2 in range(4):
                    outproj_h1_unnorm(pvs_last, h2, tt2)

    nc.compile()
    return nc


def _get_nc():
    if "nc" not in _cache:
        _cache["nc"] = _build()
    return _cache["nc"]


def _make_in_maps(x, mask, Wq, Wk, Wv, Wp):
    x = np.asarray(x, dtype=np.float32)
    mask = np.asarray(mask)
    scale = C ** (-0.5)
    wq_b = (np.asarray(Wq, np.float32) * scale).astype(BF16)
    wk_b = np.asarray(Wk, np.float32).astype(BF16)
    wv_b = np.asarray(Wv, np.float32).astype(BF16)
    wp_b = np.asarray(Wp, np.float32).astype(BF16)

    xTs, maskts = [], []
    for bi in range(B):
        xTs.append(np.ascontiguousarray(x[bi].T).astype(BF16))
        mt = (1 - mask[bi, :, 0, :]).T.astype(np.float32)  # [k, q]
        # -> [qch, kt, 128, 512]
        m4 = mt.reshape(TT, 128, NQCH, QCH).transpose(2, 0, 1, 3)
        maskts.append(np.ascontiguousarray(m4).astype(BF16))

    in_maps = []
    for core in range(NCORES):
        bi, hg = core // HPC, core % HPC
        cr = slice(256 * hg, 256 * (hg + 1))
        in_maps.append({
            "xt": xTs[bi],
            "wq": np.ascontiguousarray(wq_b[:, cr]),
            "wk": np.ascontiguousarray(wk_b[:, cr]),
            "wv": np.ascontiguousarray(wv_b[:, cr]),
            "wp": np.ascontiguousarray(wp_b[cr, :]),
            "maskt": maskts[bi],
        })
    return in_maps


def _run_once(nc, in_maps):
    _import_concourse()
    from concourse.bass_utils import run_bass_kernel_spmd

    res = run_bass_kernel_spmd(nc, in_maps, core_ids=list(range(NCORES)))
    full = np.zeros((B, N, D), np.float32)
    for core in range(NCORES):
        bi = core // HPC
        full[bi] += np.asarray(res.results[core]["out"], np.float32)
        o2 = np.asarray(res.results[core]["out2"], np.float32)
        den = np.asarray(res.results[core]["out3"],
                         np.float32).reshape(2, QCH)
        for h2 in range(2):
            full[bi, (NQCH - 1) * QCH:] += o2[h2] / den[h2][:, None]
    return full


def kernel(x, mask, Wq, Wk, Wv, Wp, bp):
    nc = _get_nc()
    in_maps = _make_in_maps(x, mask, Wq, Wk, Wv, Wp)

    # The device very occasionally returns corrupted results right after a
    # runtime error; run twice and require agreement.
    a = _run_once(nc, in_maps)
    for _ in range(3):
        b = _run_once(nc, in_maps)
        da = np.linalg.norm(a - b) / max(1e-30, np.linalg.norm(b))
        if da < 1e-4:
            break
        a = b
    full = b
    full += np.asarray(bp, np.float32)[None, None, :]
    return full


# revision 26
# speedup vs baseline: 1.0262x; 1.0262x over previous
"""Multi-head attention (AnyAttention) on 8 TRN2 NeuronCores.

Sharding: (batch, head-group): core i handles batch i//4 and heads
4*(i%4) .. 4*(i%4)+4 over ALL 2048 queries (tensor parallel on heads,
row-parallel output projection).  Each core emits a bf16 partial
output [2048, 1024] = attn_out_mine @ Wp[mine_rows]; the host sums
the 4 partials per batch in fp32.

Per-core pipeline (v2 schedule):
  - qT/kT computed c-major [128(=2 heads x 64), 2048]; v token-major
    [128 tok, 4 heads, 65] (65th col = ones -> softmax denominator
    rides the PV matmul stream)
  - logits S^T[k, q]: per (head-pair, qch=512, kt): both heads' QK
    matmuls at PE row bases 0/64 into the two banks of one
    [128, 1024] psum tile; ONE [128, 1024] exp covers both heads
  - mask applied post-exp as mul by host-prepped (1-mask)^T on
    DVE (bf16 2x mode); a few early-slot muls go to GpSimd
  - PV trails QK/exp by LAG=2 slots; the 2 leftover pairs of each
    block are CARRIED into the next block's first two slots so the
    ACT exp cadence never pauses at a block boundary
  - PV psum is double-buffered by block parity (pvA0/pvA1 vs
    pvB0/pvB1, one bank each) so the next block's PV start never
    WAR-waits on the previous block's normalize chain
  - softmax 1/denom: per head, copy the denominator row to
    partition 0, reciprocal_approx_fast [1,512], then
    gpsimd.partition_broadcast into a [64,512] bcs block; the
    normalize multiply reads PV straight out of PSUM.  No PE
    broadcast matmul and no PSUM util bank needed for finalize.
  - proj / outproj / vproj psum lives in the OPPOSITE parity's pv
    banks (free window: after that parity's finalize, before its
    next block), or the CURRENT parity's banks at pk0-1 (free until
    pv_pair(0) at pk2)
  - DMA: critical prefix wq,wk,xt (sync+scalar HW queues) before
    wv, mask0, wp so the first QK can start ~15us in
  - all matmuls bf16 with fp32 PSUM accumulation; scale 1/sqrt(c)
    folded into Wq on host; bp added on host
"""

import numpy as np
import ml_dtypes

B, N, D = 2, 2048, 1024
G, C = 16, 64          # heads, head dim
HPC = 4                # heads per core
NCORES = 8
NQCH = 4               # query chunks of 512
QCH = N // NQCH
TT = N // 128          # 16 token/key tiles
KT = D // 128          # 8 contraction tiles over d

BF16 = ml_dtypes.bfloat16

_cache = {}


def _import_concourse():
    try:
        import concourse.bass  # noqa: F401
    except ImportError:
        import sys
        sys.path.insert(0, "/opt/trn_rl_repo")


def _build():
    _import_concourse()
    import concourse.bass as bass  # noqa: F401
    from concourse import bacc, mybir
    import concourse.tile as tile

    fp32 = mybir.dt.float32
    bf16 = mybir.dt.bfloat16
    AF = mybir.ActivationFunctionType

    nc = bacc.Bacc("TRN2", target_bir_lowering=False, debug=False,
                   num_devices=NCORES)

    # ---- DRAM I/O (per-core shards; same program on all cores) ----
    xt = nc.dram_tensor("xt", [D, N], bf16, kind="ExternalInput").ap()
    wq = nc.dram_tensor("wq", [D, 256], bf16, kind="ExternalInput").ap()
    wk = nc.dram_tensor("wk", [D, 256], bf16, kind="ExternalInput").ap()
    wv = nc.dram_tensor("wv", [D, 256], bf16, kind="ExternalInput").ap()
    wp = nc.dram_tensor("wp", [256, D], bf16, kind="ExternalInput").ap()
    maskt = nc.dram_tensor("maskt", [NQCH, TT, 128, QCH], bf16,
                           kind="ExternalInput").ap()
    out3 = nc.dram_tensor("out3", [1, 2 * QCH], fp32,
                          kind="ExternalOutput").ap()
    out = nc.dram_tensor("out", [N, D], bf16, kind="ExternalOutput").ap()
    # UNNORMALIZED per-head h1 partials of the LAST qch's output
    # rows + their softmax denominators; the host normalizes and
    # adds.  Keeps the end-of-kernel drain off the whole finalize
    # chain (recip -> broadcast -> normalize).
    out2 = nc.dram_tensor("out2", [2, QCH, D], bf16,
                          kind="ExternalOutput").ap()

    GPS_MULS = 0   # h2=1 mask muls of the LAST GPS_MULS pks go to GpSimd
    # (GpSimd muls run concurrently with DVE muls on the same mask
    # slice and SBUF contention slows the DVE mul 0.6us -> 2.5us;
    # all-DVE is faster overall)

    with tile.TileContext(nc) as tc:
        with (
            tc.tile_pool(name="wts", bufs=3) as wpool,
            tc.tile_pool(name="wpp", bufs=1) as wppool,
            tc.tile_pool(name="xtp", bufs=1) as xtpool,
            tc.tile_pool(name="maskp", bufs=2) as maskpool,
            tc.tile_pool(name="stay", bufs=1) as stay,
            tc.tile_pool(name="etp", bufs=2) as etpool,
            tc.tile_pool(name="metp", bufs=3) as metpool,
            tc.tile_pool(name="bcsp", bufs=2) as bcsp,
            tc.tile_pool(name="small", bufs=1) as small,
            tc.tile_pool(name="outp", bufs=2) as outpool,
            tc.tile_pool(name="psqk", bufs=2, space="PSUM") as psqk,
            tc.tile_pool(name="pspv", bufs=1, space="PSUM") as pspv,
        ):
            # ---------- DMA: critical prefix first ----------
            # sync HW queue: wq, wk, xt evens; scalar HW queue: xt
            # odds, wv, mask0, wp.  The first QK needs wq+wk+xt; wv
            # by ~pk0 of block 0; mask0 by the first mask mul; wp
            # not until the first outproj (block 2).
            w_t = {}
            for name, dr in (("wq", wq), ("wk", wk)):
                wt = wpool.tile([128, KT * 256], bf16, tag=name, name=name)
                nc.sync.dma_start(
                    out=wt.rearrange("p (dk c) -> p dk c", dk=KT),
                    in_=dr.rearrange("(dk p) c -> p dk c", p=128))
                w_t[name] = [wt[:, dk * 256:(dk + 1) * 256]
                             for dk in range(KT)]
            xt_t = []
            for dk in range(KT):
                t = xtpool.tile([128, N], bf16, tag=f"xt{dk}")
                eng = nc.sync if dk % 2 == 0 else nc.scalar
                eng.dma_start(out=t, in_=xt[dk * 128:(dk + 1) * 128, :])
                xt_t.append(t)
            for name, dr in (("wv", wv),):
                wt = wpool.tile([128, KT * 256], bf16, tag=name, name=name)
                nc.scalar.dma_start(
                    out=wt.rearrange("p (dk c) -> p dk c", dk=KT),
                    in_=dr.rearrange("(dk p) c -> p dk c", p=128))
                w_t[name] = [wt[:, dk * 256:(dk + 1) * 256]
                             for dk in range(KT)]

            mask_t = {}

            def load_mask(qch, eng=None):
                mt = maskpool.tile([128, TT * QCH], bf16, tag="mask")
                (eng or nc.sync).dma_start(
                    out=mt.rearrange("p (kt q) -> p kt q", kt=TT),
                    in_=maskt[qch].rearrange("kt p q -> p kt q"))
                mask_t[qch] = mt

            load_mask(0, eng=nc.scalar)
            wpt = wppool.tile([128, 2 * D], bf16, tag="wp", name="wp")
            nc.scalar.dma_start(
                out=wpt.rearrange("p (r c) -> p r c", r=2),
                in_=wp.rearrange("(r p) c -> p r c", p=128))
            w_t["wp"] = [wpt[:, r * D:(r + 1) * D] for r in range(2)]
            # head-1 half of Wp's hp1 rows re-based to partition 0 for
            # the per-head unnormalized tail matmuls
            wp1b = stay.tile([C, D], bf16, tag="wp1b", name="wp1b")

            # touch Exp now so ACT_TABLE_LOAD is off the critical path
            warm = small.tile([1, C], bf16, tag="warm")
            nc.vector.memset(warm, 1.0)
            nc.scalar.activation(out=warm, in_=warm, func=AF.Exp)
            nc.vector.tensor_copy(out=wp1b,
                                  in_=wpt[C:2 * C, D:2 * D])

            qT = [stay.tile([128, N], bf16, tag=f"qT{hp}", name=f"qT{hp}")
                  for hp in range(2)]
            kTt = [stay.tile([128, N], bf16, tag=f"kT{hp}", name=f"kT{hp}")
                   for hp in range(2)]
            aT = [stay.tile([128, N], bf16, tag=f"aT{hp}", name=f"aT{hp}")
                  for hp in range(2)]
            v_t = [stay.tile([128, HPC, C + 1], bf16, tag=f"v{tt}",
                             name=f"v{tt}")
                   for tt in range(TT)]

            # ---------- PSUM: 4 banks QK + 4 banks PV (parity) ----------
            # pv tiles double as proj/outproj/vproj scratch ("util")
            # during their idle parity windows.
            def pv_tile(par, h2):
                return pspv.tile([128, QCH], fp32, tag=f"pv{par}{h2}",
                                 name=f"pv{par}{h2}")

            util_state = {"par": 0, "h2": 0}

            def util_tile():
                # rotate over the two banks of the currently-safe parity
                h2 = util_state["h2"]
                util_state["h2"] = 1 - h2
                return pv_tile(util_state["par"], h2)

            # ---------- worker emitters ----------
            def _proj_half(wname, dst, hp, ch, part, box):
                if part == 0:
                    box.clear()
                    box.append(util_tile())
                ps = box[0]
                for dk in range(part * 4, part * 4 + 4):
                    nc.tensor.matmul(
                        ps, w_t[wname][dk][:, hp * 128:(hp + 1) * 128],
                        xt_t[dk][:, ch * QCH:(ch + 1) * QCH],
                        start=(dk == 0), stop=(dk == KT - 1))
                if part == 1:
                    nc.vector.tensor_copy(
                        out=dst[hp][:, ch * QCH:(ch + 1) * QCH], in_=ps)

            def qproj_group(hp, qch, part=None, box=[]):
                if part is None:
                    _proj_half("wq", qT, hp, qch, 0, box)
                    _proj_half("wq", qT, hp, qch, 1, box)
                else:
                    _proj_half("wq", qT, hp, qch, part, box)

            def kproj_group(hp, ch, part=None, box=[]):
                if part is None:
                    _proj_half("wk", kTt, hp, ch, 0, box)
                    _proj_half("wk", kTt, hp, ch, 1, box)
                else:
                    _proj_half("wk", kTt, hp, ch, part, box)

            def vproj_group(tt):
                ps = util_tile()
                for dk in range(KT):
                    nc.tensor.matmul(
                        ps[:, 0:256],
                        xt_t[dk][:, tt * 128:(tt + 1) * 128],
                        w_t["wv"][dk],
                        start=(dk == 0), stop=(dk == KT - 1))
                vt = v_t[tt]
                nc.vector.memset(vt[:, :, C:C + 1], 1.0)
                nc.vector.tensor_copy(
                    out=vt[:, :, 0:C],
                    in_=ps[:, 0:256].rearrange("p (h c) -> p h c", c=C))

            def outproj_h0_slab(tt2):
                # normal h0 output rows for the LAST qch
                r0 = (NQCH - 1) * QCH + tt2 * 128
                osb = outpool.tile([128, D], bf16, tag="osb")
                for dch in range(2):
                    ps = util_tile()
                    nc.tensor.matmul(
                        ps, aT[0][:, r0:r0 + 128],
                        w_t["wp"][0][:, dch * QCH:(dch + 1) * QCH],
                        start=True, stop=True)
                    nc.vector.tensor_copy(
                        out=osb[:, dch * QCH:(dch + 1) * QCH], in_=ps)
                nc.sync.dma_start(out=out[r0:r0 + 128, :], in_=osb)

            def outproj_h1_unnorm(pvs, h2, tt2):
                # per-head UNNORMALIZED h1 output rows of the last
                # qch, straight from the pvs cast (no finalize wait)
                osb = outpool.tile([128, D], bf16, tag="osb")
                wphalf = w_t["wp"][1] if h2 == 0 else None
                for dch in range(2):
                    ps = util_tile()
                    if h2 == 0:
                        rhs = w_t["wp"][1][0:C,
                                           dch * QCH:(dch + 1) * QCH]
                    else:
                        rhs = wp1b[:, dch * QCH:(dch + 1) * QCH]
                    nc.tensor.matmul(
                        ps[:, :],
                        pvs[:, h2 * QCH + tt2 * 128:
                            h2 * QCH + (tt2 + 1) * 128],
                        rhs, start=True, stop=True)
                    nc.vector.tensor_copy(
                        out=osb[:, dch * QCH:(dch + 1) * QCH], in_=ps)
                nc.sync.dma_start(
                    out=out2[h2, tt2 * 128:(tt2 + 1) * 128, :], in_=osb)

            osb_box = {}

            def outproj_half(qch, tt2, dch):
                # one dch half of one 128-token output row block; the
                # DMA fires with the second half
                if dch == 0:
                    osb_box[tt2] = outpool.tile([128, D], bf16,
                                                tag="osb", name="osb")
                osb = osb_box[tt2]
                r0 = qch * QCH + tt2 * 128
                ps = util_tile()
                for hp in range(2):
                    nc.tensor.matmul(
                        ps, aT[hp][:, r0:r0 + 128],
                        w_t["wp"][hp][:, dch * QCH:(dch + 1) * QCH],
                        start=(hp == 0), stop=(hp == 1))
                nc.vector.tensor_copy(
                    out=osb[:, dch * QCH:(dch + 1) * QCH], in_=ps)
                if dch == 1:
                    nc.sync.dma_start(out=out[r0:r0 + 128, :], in_=osb)

            # ---------- attention block for (qch, hp) ----------
            # PV trails the unit mask-mul by a full slot (the mul for
            # unit u lands end of pk 2u+1; pv_pair(2u) runs at 2u+3)
            LAG = 3

            def attn_block(blk, qch, hp, interleave, carry_in,
                           last=False):
                par = blk % 2
                mt = mask_t[qch]
                pv = [pv_tile(par, h2)[0:C + 1, :] for h2 in range(2)]
                # et/met tiles cover a 2-pair UNIT (4 kt): the mask
                # multiply is then ONE out-of-place [128,2048] bf16
                # tensor_tensor per head per unit, which the DVE runs
                # in 4x mode (~690ns vs 2x 601ns per [128,1024])
                et_units = [None] * (TT // 4)
                met_units = [None] * (TT // 4)

                def pv_pair(pk2, pv=pv, hp=hp):
                    metu = met_units[pk2 // 2]
                    pr = pk2 % 2
                    for j in range(2):
                        kt = 2 * pk2 + j
                        for h2 in range(2):
                            nc.tensor.matmul(
                                pv[h2], v_t[kt][:, hp * 2 + h2, :],
                                metu[:, h2, pr, j, :],
                                start=(kt == 0), stop=(kt == TT - 1))

                for pk in range(TT // 2):
                    u, pr = pk // 2, pk % 2
                    if pr == 0:
                        et_units[u] = etpool.tile(
                            [128, 2, 2, 2, QCH], bf16, tag="et",
                            name="et")
                        met_units[u] = metpool.tile(
                            [128, 2, 2, 2, QCH], bf16, tag="met",
                            name="met")
                    et = et_units[u]
                    for j in ((0, 1) if pk % 2 == 0 else (1, 0)):
                        kt = 2 * pk + j
                        ps = psqk.tile([128, 2 * QCH], fp32, tag="qk")
                        for h2 in range(2):
                            pb = h2 * C
                            nc.tensor.matmul(
                                ps[:, h2 * QCH:(h2 + 1) * QCH],
                                kTt[hp][pb:pb + C,
                                        kt * 128:(kt + 1) * 128],
                                qT[hp][pb:pb + C,
                                       qch * QCH:(qch + 1) * QCH],
                                start=True, stop=True)
                        nc.scalar.activation(
                            out=et[:, :, pr, j, :],
                            in_=ps.rearrange("p (h q) -> p h q", h=2),
                            func=AF.Exp)
                    # carry first: its PE ops (tail pv pairs, pk0-1)
                    # depend only on the previous block's tiles, and
                    # its DVE ops (finalize stage 1, pk2) must beat
                    # this slot's mask muls into the DVE FIFO so the
                    # pv banks free on time
                    if carry_in is not None and pk < len(carry_in):
                        carry_in[pk]()
                    if pr == 1:
                        met = met_units[u]
                        msl = mt[:, u * 4 * QCH:(u + 1) * 4 * QCH]
                        for h2 in range(2):
                            nc.vector.tensor_mul(
                                met[:, h2].rearrange(
                                    "p a b c -> p (a b c)"),
                                et[:, h2].rearrange(
                                    "p a b c -> p (a b c)"),
                                msl)
                    if pk >= LAG:
                        pv_pair(pk - LAG)
                    for f in interleave.get(pk, ()):
                        f()

                # Finalize, staged across carry slots so no DVE op
                # ever waits in-FIFO on the gpsimd broadcast:
                #   pk2: denominator rows -> [1,1024] row, pv value
                #        rows -> SBUF bf16 (frees the pv psum banks)
                #   pk3: recip [1,1024] + bf16 cast + gpsimd
                #        broadcast to [64,1024]
                #   pk5: 2x-mode bf16 normalize multiplies into aT
                fin_box = {}

                def fin_stage1(pv=pv):
                    # recip chain first (it gates the bc matmuls at
                    # pk3), then the pv value casts (they gate only
                    # the pk4+ norm muls / bank release)
                    dga = small.tile([1, 2 * QCH], fp32, tag="dga")
                    for h2 in range(2):
                        nc.vector.tensor_copy(
                            out=dga[:, h2 * QCH:(h2 + 1) * QCH],
                            in_=pv[h2][C:C + 1, :])
                    rcf = small.tile([1, 2 * QCH], fp32, tag="rcf")
                    nc.vector.reciprocal_approx_fast(rcf, dga)
                    rcb = small.tile([1, 2 * QCH], bf16, tag="rcb")
                    nc.vector.tensor_copy(out=rcb, in_=rcf)
                    fin_box["rcb"] = rcb
                    pvs = bcsp.tile([C, 2 * QCH], bf16, tag="pvs",
                                    name="pvs")
                    for h2 in range(2):
                        nc.vector.tensor_copy(
                            out=pvs[:, h2 * QCH:(h2 + 1) * QCH],
                            in_=pv[h2][0:C, :])
                    fin_box["pvs"] = pvs

                def fin_bc():
                    # broadcast the recip row to 64 partitions on the
                    # (otherwise idle) GpSimd -- its library never
                    # switches, so no reload cost
                    bcs = bcsp.tile([C, 2 * QCH], bf16, tag="bcs",
                                    name="bcs")
                    nc.gpsimd.partition_broadcast(bcs, fin_box["rcb"])
                    fin_box["bcs"] = bcs

                def fin_norm(hp=hp, qch=qch):
                    pvs, bcs = fin_box["pvs"], fin_box["bcs"]
                    for h2 in range(2):
                        nc.vector.tensor_mul(
                            aT[hp][h2 * C:(h2 + 1) * C,
                                   qch * QCH:(qch + 1) * QCH],
                            pvs[:, h2 * QCH:(h2 + 1) * QCH],
                            bcs[:, h2 * QCH:(h2 + 1) * QCH])

                # carry closures: the LAG leftover pv pairs + the
                # staged finalize, woven into the NEXT block's slots
                def tail0():
                    pv_pair(TT // 2 - LAG)

                def tail1():
                    pv_pair(TT // 2 - LAG + 1)

                def tail2():
                    pv_pair(TT // 2 - LAG + 2)

                if last:
                    # no on-chip normalize for the final block: ship
                    # raw denominators + unnormalized pvs
                    def last_fin(pv=pv):
                        dga = small.tile([1, 2 * QCH], fp32, tag="dga")
                        pvs = bcsp.tile([C, 2 * QCH], bf16, tag="pvs",
                                        name="pvs")
                        for h2 in range(2):
                            nc.vector.tensor_copy(
                                out=dga[:, h2 * QCH:(h2 + 1) * QCH],
                                in_=pv[h2][C:C + 1, :])
                            nc.vector.tensor_copy(
                                out=pvs[:, h2 * QCH:(h2 + 1) * QCH],
                                in_=pv[h2][0:C, :])
                        nc.scalar.dma_start(out=out3, in_=dga)
                        fin_box["pvs"] = pvs

                    return [tail0, tail1, tail2, last_fin], fin_box
                return [tail0, tail1, tail2, fin_stage1, fin_bc,
                        fin_norm]

            # ---------- emission schedule ----------
            # prologue: kproj(0, ch0) + qproj(0, qch0) halves
            # interleaved so both accumulation chains track the
            # per-dk xt DMA arrivals
            util_state["par"] = 1   # block 0 (par 0) borrows pvB
            bq_p, bk_p = [], []
            qproj_group(0, 0, 0, bq_p)
            kproj_group(0, 0, 0, bk_p)
            qproj_group(0, 0, 1, bq_p)
            kproj_group(0, 0, 1, bk_p)

            carry = None
            blk = 0
            for qch in range(NQCH):
                if qch + 1 < NQCH:
                    load_mask(qch + 1,
                              eng=(nc.sync if qch % 2 == 0 else nc.scalar))
                for hp in range(2):
                    par = blk % 2
                    il = {}

                    def add(pk, f):
                        il.setdefault(pk, []).append(f)

                    # util banks: pk0-2 -> current parity (free until
                    # pv_pair(0) at pk3); pk3+ -> opposite parity
                    # (freed by the carried finalize stage1 at pk3)
                    def set_util(p):
                        def f(p=p):
                            util_state["par"] = p
                        return f

                    add(0, set_util(par))
                    add(3, set_util(1 - par))

                    if qch == 0 and hp == 0:
                        # v tiles (2p, 2p+1) consumed by pv_pair(p) at
                        # pk=p+2 (tails p=6,7 in the next block's
                        # pk0,1); k chunks ch needed by QK at pk=2ch.
                        # NOTE pk2 is a util-bank hole: own-parity
                        # banks start accumulating PV at pk2, opposite
                        # banks are read by the carried finalize until
                        # ~pk3.
                        add(0, lambda: vproj_group(0))
                        add(0, lambda: vproj_group(1))
                        add(0, lambda: kproj_group(0, 1))
                        add(1, lambda: vproj_group(2))
                        add(1, lambda: vproj_group(3))
                        add(1, lambda: kproj_group(0, 2))
                        add(2, lambda: vproj_group(4))
                        add(2, lambda: vproj_group(5))
                        add(3, lambda: vproj_group(6))
                        add(3, lambda: vproj_group(7))
                        add(3, lambda: kproj_group(0, 3))
                        add(4, lambda: vproj_group(8))
                        add(4, lambda: vproj_group(9))
                        add(5, lambda: vproj_group(10))
                        add(5, lambda: vproj_group(11))
                        add(5, lambda: kproj_group(1, 0))
                        add(6, lambda: vproj_group(12))
                        add(6, lambda: vproj_group(13))
                        bq1 = []
                        add(6, lambda: qproj_group(1, 0, 0, bq1))
                        add(7, lambda: vproj_group(14))
                        add(7, lambda: vproj_group(15))
                        add(7, lambda: qproj_group(1, 0, 1, bq1))
                    if qch == 0 and hp == 1:
                        b1, b2, b3 = [], [], []
                        add(0, lambda: kproj_group(1, 1, 0, b1))
                        add(1, lambda: kproj_group(1, 1, 1, b1))
                        add(1, lambda: kproj_group(1, 2, 0, b2))
                        add(2, lambda: kproj_group(1, 2, 1, b2))
                        add(4, lambda: kproj_group(1, 3, 0, b3))
                        add(5, lambda: kproj_group(1, 3, 1, b3))
                    if hp == 0 and qch > 0:
                        # qproj for this qch's hp1 block (needed by its
                        # pk0); opposite banks freed by stage1 ~pk4.2
                        bq = []
                        add(4, lambda q=qch: qproj_group(1, q, 0, bq))
                        add(5, lambda q=qch: qproj_group(1, q, 1, bq))
                        # output projection of the previous qch, in
                        # dch-halves; its aT lands ~pk5.6 (norm at pk5)
                        add(6, lambda q=qch - 1: outproj_half(q, 0, 0))
                        add(7, lambda q=qch - 1: outproj_half(q, 0, 1))
                    if hp == 1 and qch > 0:
                        # pk0-2 use this block's OWN banks (pv starts
                        # at pk3)
                        add(0, lambda q=qch - 1: outproj_half(q, 1, 0))
                        add(1, lambda q=qch - 1: outproj_half(q, 1, 1))
                        add(2, lambda q=qch - 1: outproj_half(q, 2, 0))
                        add(4, lambda q=qch - 1: outproj_half(q, 2, 1))
                        add(6, lambda q=qch - 1: outproj_half(q, 3, 0))
                        add(7, lambda q=qch - 1: outproj_half(q, 3, 1))
                    if hp == 1 and qch + 1 < NQCH:
                        bq0 = []
                        add(4, lambda q=qch: qproj_group(0, q + 1, 0, bq0))
                        add(5, lambda q=qch: qproj_group(0, q + 1, 1, bq0))
                    if hp == 1 and qch == NQCH - 1:
                        # aT[0][q3] lands ~pk5.6 (finalize of (q3,h0)
                        # carried into this block)
                        add(6, lambda: outproj_h0_slab(0))
                        add(7, lambda: outproj_h0_slab(1))
                    last = (qch == NQCH - 1 and hp == 1)
                    r = attn_block(blk, qch, hp, il, carry, last=last)
                    if last:
                        carry, last_box = r
                    else:
                        carry = r
                    blk += 1
            # last block: tails + raw-denominator export, then the
            # unnormalized per-head h1 output rows (no finalize wait)
            for f in carry:
                f()
            util_state["par"] = 0   # last block was par 1
            outproj_h0_slab(2)
            outproj_h0_slab(3)
            pvs_last = last_box["pvs"]
            for h2 in range(2):
                for tt2 in range(4):
                    outproj_h1_unnorm(pvs_last, h2, tt2)

    nc.compile()
    return nc


def _get_nc():
    if "nc" not in _cache:
        _cache["nc"] = _build()
    return _cache["nc"]


def _make_in_maps(x, mask, Wq, Wk, Wv, Wp):
    x = np.asarray(x, dtype=np.float32)
    mask = np.asarray(mask)
    scale = C ** (-0.5)
    wq_b = (np.asarray(Wq, np.float32) * scale).astype(BF16)
    wk_b = np.asarray(Wk, np.float32).astype(BF16)
    wv_b = np.asarray(Wv, np.float32).astype(BF16)
    wp_b = np.asarray(Wp, np.float32).astype(BF16)

    xTs, maskts = [], []
    for bi in range(B):
        xTs.append(np.ascontiguousarray(x[bi].T).astype(BF16))
        mt = (1 - mask[bi, :, 0, :]).T.astype(np.float32)  # [k, q]
        # -> [qch, kt, 128, 512]
        m4 = mt.reshape(TT, 128, NQCH, QCH).transpose(2, 0, 1, 3)
        maskts.append(np.ascontiguousarray(m4).astype(BF16))

    in_maps = []
    for core in range(NCORES):
        bi, hg = core // HPC, core % HPC
        cr = slice(256 * hg, 256 * (hg + 1))
        in_maps.append({
            "xt": xTs[bi],
            "wq": np.ascontiguousarray(wq_b[:, cr]),
            "wk": np.ascontiguousarray(wk_b[:, cr]),
            "wv": np.ascontiguousarray(wv_b[:, cr]),
            "wp": np.ascontiguousarray(wp_b[cr, :]),
            "maskt": maskts[bi],
        })
    return in_maps


def _run_once(nc, in_maps):
    _import_concourse()
    from concourse.bass_utils import run_bass_kernel_spmd

    res = run_bass_kernel_spmd(nc, in_maps, core_ids=list(range(NCORES)))
    full = np.zeros((B, N, D), np.float32)
    for core in range(NCORES):
        bi = core // HPC
        full[bi] += np.asarray(res.results[core]["out"], np.float32)
        o2 = np.asarray(res.results[core]["out2"], np.float32)
        den = np.asarray(res.results[core]["out3"],
                         np.float32).reshape(2, QCH)
        for h2 in range(2):
            full[bi, (NQCH - 1) * QCH:] += o2[h2] / den[h2][:, None]
    return full


def kernel(x, mask, Wq, Wk, Wv, Wp, bp):
    nc = _get_nc()
    in_maps = _make_in_maps(x, mask, Wq, Wk, Wv, Wp)

    # The device very occasionally returns corrupted results right after a
    # runtime error; run twice and require agreement.
    a = _run_once(nc, in_maps)
    for _ in range(3):
        b = _run_once(nc, in_maps)
        da = np.linalg.norm(a - b) / max(1e-30, np.linalg.norm(b))
        if da < 1e-4:
            break
        a = b
    full = b
    full += np.asarray(bp, np.float32)[None, None, :]
    return full


# revision 27
# speedup vs baseline: 1.0924x; 1.0645x over previous
"""Multi-head attention (AnyAttention) on 8 TRN2 NeuronCores.

Sharding: (batch, head-group): core i handles batch i//4 and heads
4*(i%4) .. 4*(i%4)+4 over ALL 2048 queries (tensor parallel on heads,
row-parallel output projection).  Each core emits a bf16 partial
output [2048, 1024] = attn_out_mine @ Wp[mine_rows]; the host sums
the 4 partials per batch in fp32.

Per-core pipeline (v2 schedule):
  - qT/kT computed c-major [128(=2 heads x 64), 2048]; v token-major
    [128 tok, 4 heads, 65] (65th col = ones -> softmax denominator
    rides the PV matmul stream)
  - logits S^T[k, q]: per (head-pair, qch=512, kt): both heads' QK
    matmuls at PE row bases 0/64 into the two banks of one
    [128, 1024] psum tile; ONE [128, 1024] exp covers both heads
  - mask applied post-exp as mul by host-prepped (1-mask)^T on
    DVE (bf16 2x mode); a few early-slot muls go to GpSimd
  - PV trails QK/exp by LAG=2 slots; the 2 leftover pairs of each
    block are CARRIED into the next block's first two slots so the
    ACT exp cadence never pauses at a block boundary
  - PV psum is double-buffered by block parity (pvA0/pvA1 vs
    pvB0/pvB1, one bank each) so the next block's PV start never
    WAR-waits on the previous block's normalize chain
  - softmax 1/denom: per head, copy the denominator row to
    partition 0, reciprocal_approx_fast [1,512], then
    gpsimd.partition_broadcast into a [64,512] bcs block; the
    normalize multiply reads PV straight out of PSUM.  No PE
    broadcast matmul and no PSUM util bank needed for finalize.
  - proj / outproj / vproj psum lives in the OPPOSITE parity's pv
    banks (free window: after that parity's finalize, before its
    next block), or the CURRENT parity's banks at pk0-1 (free until
    pv_pair(0) at pk2)
  - DMA: critical prefix wq,wk,xt (sync+scalar HW queues) before
    wv, mask0, wp so the first QK can start ~15us in
  - all matmuls bf16 with fp32 PSUM accumulation; scale 1/sqrt(c)
    folded into Wq on host; bp added on host
"""

import numpy as np
import ml_dtypes

B, N, D = 2, 2048, 1024
G, C = 16, 64          # heads, head dim
HPC = 4                # heads per core
NCORES = 8
NQCH = 4               # query chunks of 512
QCH = N // NQCH
TT = N // 128          # 16 token/key tiles
KT = D // 128          # 8 contraction tiles over d

BF16 = ml_dtypes.bfloat16

_cache = {}


def _import_concourse():
    try:
        import concourse.bass  # noqa: F401
    except ImportError:
        import sys
        sys.path.insert(0, "/opt/trn_rl_repo")


def _build():
    _import_concourse()
    import concourse.bass as bass  # noqa: F401
    from concourse import bacc, mybir
    import concourse.tile as tile

    fp32 = mybir.dt.float32
    bf16 = mybir.dt.bfloat16
    AF = mybir.ActivationFunctionType

    nc = bacc.Bacc("TRN2", target_bir_lowering=False, debug=False,
                   num_devices=NCORES)

    # ---- DRAM I/O (per-core shards; same program on all cores) ----
    xt = nc.dram_tensor("xt", [D, N], bf16, kind="ExternalInput").ap()
    wq = nc.dram_tensor("wq", [D, 256], bf16, kind="ExternalInput").ap()
    wk = nc.dram_tensor("wk", [D, 256], bf16, kind="ExternalInput").ap()
    wv = nc.dram_tensor("wv", [D, 256], bf16, kind="ExternalInput").ap()
    wp = nc.dram_tensor("wp", [256, D], bf16, kind="ExternalInput").ap()
    maskt = nc.dram_tensor("maskt", [NQCH, TT, 128, QCH], bf16,
                           kind="ExternalInput").ap()
    out3 = nc.dram_tensor("out3", [1, 2 * QCH], fp32,
                          kind="ExternalOutput").ap()
    out = nc.dram_tensor("out", [N, D], bf16, kind="ExternalOutput").ap()
    # UNNORMALIZED per-head h1 partials of the LAST qch's output
    # rows + their softmax denominators; the host normalizes and
    # adds.  Keeps the end-of-kernel drain off the whole finalize
    # chain (recip -> broadcast -> normalize).
    out2 = nc.dram_tensor("out2", [2, QCH, D], bf16,
                          kind="ExternalOutput").ap()

    GPS_MULS = 0   # h2=1 mask muls of the LAST GPS_MULS pks go to GpSimd
    # (GpSimd muls run concurrently with DVE muls on the same mask
    # slice and SBUF contention slows the DVE mul 0.6us -> 2.5us;
    # all-DVE is faster overall)

    with tile.TileContext(nc) as tc:
        with (
            tc.tile_pool(name="wts", bufs=3) as wpool,
            tc.tile_pool(name="wpp", bufs=1) as wppool,
            tc.tile_pool(name="xtp", bufs=1) as xtpool,
            tc.tile_pool(name="maskp", bufs=2) as maskpool,
            tc.tile_pool(name="stay", bufs=1) as stay,
            tc.tile_pool(name="etp", bufs=2) as etpool,
            tc.tile_pool(name="metp", bufs=3) as metpool,
            tc.tile_pool(name="bcsp", bufs=2) as bcsp,
            tc.tile_pool(name="small", bufs=1) as small,
            tc.tile_pool(name="outp", bufs=2) as outpool,
            tc.tile_pool(name="psqk", bufs=2, space="PSUM") as psqk,
            tc.tile_pool(name="pspv", bufs=1, space="PSUM") as pspv,
        ):
            # ---------- DMA: critical prefix first ----------
            # sync HW queue: wq, wk, xt evens; scalar HW queue: xt
            # odds, wv, mask0, wp.  The first QK needs wq+wk+xt; wv
            # by ~pk0 of block 0; mask0 by the first mask mul; wp
            # not until the first outproj (block 2).
            w_t = {}
            for name, dr in (("wq", wq), ("wk", wk)):
                wt = wpool.tile([128, KT * 256], bf16, tag=name, name=name)
                nc.sync.dma_start(
                    out=wt.rearrange("p (dk c) -> p dk c", dk=KT),
                    in_=dr.rearrange("(dk p) c -> p dk c", p=128))
                w_t[name] = [wt[:, dk * 256:(dk + 1) * 256]
                             for dk in range(KT)]
            xt_t = []
            for dk in range(KT):
                t = xtpool.tile([128, N], bf16, tag=f"xt{dk}")
                eng = nc.sync if dk % 2 == 0 else nc.scalar
                eng.dma_start(out=t, in_=xt[dk * 128:(dk + 1) * 128, :])
                xt_t.append(t)
            for name, dr in (("wv", wv),):
                wt = wpool.tile([128, KT * 256], bf16, tag=name, name=name)
                nc.scalar.dma_start(
                    out=wt.rearrange("p (dk c) -> p dk c", dk=KT),
                    in_=dr.rearrange("(dk p) c -> p dk c", p=128))
                w_t[name] = [wt[:, dk * 256:(dk + 1) * 256]
                             for dk in range(KT)]

            mask_t = {}

            def load_mask(qch, eng=None):
                mt = maskpool.tile([128, TT * QCH], bf16, tag="mask")
                (eng or nc.sync).dma_start(
                    out=mt.rearrange("p (kt q) -> p kt q", kt=TT),
                    in_=maskt[qch].rearrange("kt p q -> p kt q"))
                mask_t[qch] = mt

            load_mask(0, eng=nc.scalar)
            wpt = wppool.tile([128, 2 * D], bf16, tag="wp", name="wp")
            nc.scalar.dma_start(
                out=wpt.rearrange("p (r c) -> p r c", r=2),
                in_=wp.rearrange("(r p) c -> p r c", p=128))
            w_t["wp"] = [wpt[:, r * D:(r + 1) * D] for r in range(2)]
            # head-1 half of Wp's hp1 rows re-based to partition 0 for
            # the per-head unnormalized tail matmuls
            wp1b = stay.tile([C, D], bf16, tag="wp1b", name="wp1b")

            # touch Exp now so ACT_TABLE_LOAD is off the critical path
            warm = small.tile([1, C], bf16, tag="warm")
            nc.vector.memset(warm, 1.0)
            nc.scalar.activation(out=warm, in_=warm, func=AF.Exp)
            nc.vector.tensor_copy(out=wp1b,
                                  in_=wpt[C:2 * C, D:2 * D])

            qT = [stay.tile([128, N], bf16, tag=f"qT{hp}", name=f"qT{hp}")
                  for hp in range(2)]
            kTt = [stay.tile([128, N], bf16, tag=f"kT{hp}", name=f"kT{hp}")
                   for hp in range(2)]
            aT = [stay.tile([128, N], bf16, tag=f"aT{hp}", name=f"aT{hp}")
                  for hp in range(2)]
            v_t = [stay.tile([128, HPC, C + 1], bf16, tag=f"v{tt}",
                             name=f"v{tt}")
                   for tt in range(TT)]

            # ---------- PSUM: 4 banks QK + 4 banks PV (parity) ----------
            # pv tiles double as proj/outproj/vproj scratch ("util")
            # during their idle parity windows.
            def pv_tile(par, h2):
                return pspv.tile([128, QCH], fp32, tag=f"pv{par}{h2}",
                                 name=f"pv{par}{h2}")

            util_state = {"par": 0, "h2": 0}

            def util_tile():
                # rotate over the two banks of the currently-safe parity
                h2 = util_state["h2"]
                util_state["h2"] = 1 - h2
                return pv_tile(util_state["par"], h2)

            # ---------- worker emitters ----------
            def _copy(eng, out, in_):
                # ACT 'copy' absorbs PSUM->SBUF casts on slots where
                # the exp pipeline is idle anyway (blocks 0-1, tail)
                if eng is nc.scalar:
                    nc.scalar.copy(out=out, in_=in_)
                else:
                    nc.vector.tensor_copy(out=out, in_=in_)

            def _proj_half(wname, dst, hp, ch, part, box, cp=None):
                if part == 0:
                    box.clear()
                    box.append(util_tile())
                ps = box[0]
                for dk in range(part * 4, part * 4 + 4):
                    nc.tensor.matmul(
                        ps, w_t[wname][dk][:, hp * 128:(hp + 1) * 128],
                        xt_t[dk][:, ch * QCH:(ch + 1) * QCH],
                        start=(dk == 0), stop=(dk == KT - 1))
                if part == 1:
                    _copy(cp or nc.vector,
                          dst[hp][:, ch * QCH:(ch + 1) * QCH], ps)

            def qproj_group(hp, qch, part=None, box=[], cp=None):
                if part is None:
                    _proj_half("wq", qT, hp, qch, 0, box, cp)
                    _proj_half("wq", qT, hp, qch, 1, box, cp)
                else:
                    _proj_half("wq", qT, hp, qch, part, box, cp)

            def kproj_group(hp, ch, part=None, box=[], cp=None):
                if part is None:
                    _proj_half("wk", kTt, hp, ch, 0, box, cp)
                    _proj_half("wk", kTt, hp, ch, 1, box, cp)
                else:
                    _proj_half("wk", kTt, hp, ch, part, box, cp)

            def vproj_group(tt):
                ps = util_tile()
                for dk in range(KT):
                    nc.tensor.matmul(
                        ps[:, 0:256],
                        xt_t[dk][:, tt * 128:(tt + 1) * 128],
                        w_t["wv"][dk],
                        start=(dk == 0), stop=(dk == KT - 1))
                vt = v_t[tt]
                nc.vector.memset(vt[:, :, C:C + 1], 1.0)
                nc.scalar.copy(
                    out=vt[:, :, 0:C],
                    in_=ps[:, 0:256].rearrange("p (h c) -> p h c", c=C))

            def outproj_h0_slab(tt2, tail=False):
                # normal h0 output rows for the LAST qch
                r0 = (NQCH - 1) * QCH + tt2 * 128
                osb = outpool.tile([128, D], bf16, tag="osb")
                for dch in range(2):
                    ps = util_tile()
                    nc.tensor.matmul(
                        ps, aT[0][:, r0:r0 + 128],
                        w_t["wp"][0][:, dch * QCH:(dch + 1) * QCH],
                        start=True, stop=True)
                    _copy(nc.scalar if (tail and dch == 0) else nc.vector,
                          osb[:, dch * QCH:(dch + 1) * QCH], ps)
                nc.sync.dma_start(out=out[r0:r0 + 128, :], in_=osb)

            def outproj_h1_unnorm(pvs, h2, tt2):
                # per-head UNNORMALIZED h1 output rows of the last
                # qch, straight from the pvs cast (no finalize wait);
                # casts split between DVE and the now-idle ACT
                osb = outpool.tile([128, D], bf16, tag="osb")
                for dch in range(2):
                    ps = util_tile()
                    if h2 == 0:
                        rhs = w_t["wp"][1][0:C,
                                           dch * QCH:(dch + 1) * QCH]
                    else:
                        rhs = wp1b[:, dch * QCH:(dch + 1) * QCH]
                    nc.tensor.matmul(
                        ps[:, :],
                        pvs[:, h2 * QCH + tt2 * 128:
                            h2 * QCH + (tt2 + 1) * 128],
                        rhs, start=True, stop=True)
                    _copy(nc.scalar if dch == 0 else nc.vector,
                          osb[:, dch * QCH:(dch + 1) * QCH], ps)
                eng = nc.sync if (h2 * 4 + tt2) % 2 == 0 else nc.scalar
                eng.dma_start(
                    out=out2[h2, tt2 * 128:(tt2 + 1) * 128, :], in_=osb)

            osb_box = {}

            def outproj_half(qch, tt2, dch):
                # one dch half of one 128-token output row block; the
                # DMA fires with the second half
                if dch == 0:
                    osb_box[tt2] = outpool.tile([128, D], bf16,
                                                tag="osb", name="osb")
                osb = osb_box[tt2]
                r0 = qch * QCH + tt2 * 128
                ps = util_tile()
                for hp in range(2):
                    nc.tensor.matmul(
                        ps, aT[hp][:, r0:r0 + 128],
                        w_t["wp"][hp][:, dch * QCH:(dch + 1) * QCH],
                        start=(hp == 0), stop=(hp == 1))
                nc.vector.tensor_copy(
                    out=osb[:, dch * QCH:(dch + 1) * QCH], in_=ps)
                if dch == 1:
                    nc.sync.dma_start(out=out[r0:r0 + 128, :], in_=osb)

            # ---------- attention block for (qch, hp) ----------
            # PV trails the unit mask-mul by a full slot (the mul for
            # unit u lands end of pk 2u+1; pv_pair(2u) runs at 2u+3)
            LAG = 3

            def attn_block(blk, qch, hp, interleave, carry_in,
                           last=False):
                par = blk % 2
                mt = mask_t[qch]
                pv = [pv_tile(par, h2)[0:C + 1, :] for h2 in range(2)]
                # et/met tiles cover a 2-pair UNIT (4 kt): the mask
                # multiply is then ONE out-of-place [128,2048] bf16
                # tensor_tensor per head per unit, which the DVE runs
                # in 4x mode (~690ns vs 2x 601ns per [128,1024])
                et_units = [None] * (TT // 4)
                met_units = [None] * (TT // 4)

                def pv_pair(pk2, pv=pv, hp=hp):
                    metu = met_units[pk2 // 2]
                    pr = pk2 % 2
                    for j in range(2):
                        kt = 2 * pk2 + j
                        for h2 in range(2):
                            nc.tensor.matmul(
                                pv[h2], v_t[kt][:, hp * 2 + h2, :],
                                metu[:, h2, pr, j, :],
                                start=(kt == 0), stop=(kt == TT - 1))

                for pk in range(TT // 2):
                    u, pr = pk // 2, pk % 2
                    if pr == 0:
                        et_units[u] = etpool.tile(
                            [128, 2, 2, 2, QCH], bf16, tag="et",
                            name="et")
                        met_units[u] = metpool.tile(
                            [128, 2, 2, 2, QCH], bf16, tag="met",
                            name="met")
                    et = et_units[u]
                    for j in ((0, 1) if pk % 2 == 0 else (1, 0)):
                        kt = 2 * pk + j
                        ps = psqk.tile([128, 2 * QCH], fp32, tag="qk")
                        for h2 in range(2):
                            pb = h2 * C
                            nc.tensor.matmul(
                                ps[:, h2 * QCH:(h2 + 1) * QCH],
                                kTt[hp][pb:pb + C,
                                        kt * 128:(kt + 1) * 128],
                                qT[hp][pb:pb + C,
                                       qch * QCH:(qch + 1) * QCH],
                                start=True, stop=True)
                        nc.scalar.activation(
                            out=et[:, :, pr, j, :],
                            in_=ps.rearrange("p (h q) -> p h q", h=2),
                            func=AF.Exp)
                    # carry first: its PE ops (tail pv pairs, pk0-1)
                    # depend only on the previous block's tiles, and
                    # its DVE ops (finalize stage 1, pk2) must beat
                    # this slot's mask muls into the DVE FIFO so the
                    # pv banks free on time
                    if carry_in is not None and pk < len(carry_in):
                        carry_in[pk]()
                    if pr == 1:
                        met = met_units[u]
                        msl = mt[:, u * 4 * QCH:(u + 1) * 4 * QCH]
                        for h2 in range(2):
                            nc.vector.tensor_mul(
                                met[:, h2].rearrange(
                                    "p a b c -> p (a b c)"),
                                et[:, h2].rearrange(
                                    "p a b c -> p (a b c)"),
                                msl)
                    if pk >= LAG:
                        pv_pair(pk - LAG)
                    for f in interleave.get(pk, ()):
                        f()

                # Finalize, staged across carry slots so no DVE op
                # ever waits in-FIFO on the gpsimd broadcast:
                #   pk2: denominator rows -> [1,1024] row, pv value
                #        rows -> SBUF bf16 (frees the pv psum banks)
                #   pk3: recip [1,1024] + bf16 cast + gpsimd
                #        broadcast to [64,1024]
                #   pk5: 2x-mode bf16 normalize multiplies into aT
                fin_box = {}

                def fin_stage1(pv=pv):
                    # recip chain first (it gates the bc matmuls at
                    # pk3), then the pv value casts (they gate only
                    # the pk4+ norm muls / bank release)
                    dga = small.tile([1, 2 * QCH], fp32, tag="dga")
                    for h2 in range(2):
                        nc.vector.tensor_copy(
                            out=dga[:, h2 * QCH:(h2 + 1) * QCH],
                            in_=pv[h2][C:C + 1, :])
                    rcf = small.tile([1, 2 * QCH], fp32, tag="rcf")
                    nc.vector.reciprocal_approx_fast(rcf, dga)
                    rcb = small.tile([1, 2 * QCH], bf16, tag="rcb")
                    nc.vector.tensor_copy(out=rcb, in_=rcf)
                    fin_box["rcb"] = rcb
                    pvs = bcsp.tile([C, 2 * QCH], bf16, tag="pvs",
                                    name="pvs")
                    for h2 in range(2):
                        nc.vector.tensor_copy(
                            out=pvs[:, h2 * QCH:(h2 + 1) * QCH],
                            in_=pv[h2][0:C, :])
                    fin_box["pvs"] = pvs

                def fin_bc():
                    # broadcast the recip row to 64 partitions on the
                    # (otherwise idle) GpSimd -- its library never
                    # switches, so no reload cost
                    bcs = bcsp.tile([C, 2 * QCH], bf16, tag="bcs",
                                    name="bcs")
                    nc.gpsimd.partition_broadcast(bcs, fin_box["rcb"])
                    fin_box["bcs"] = bcs

                def fin_norm(hp=hp, qch=qch):
                    pvs, bcs = fin_box["pvs"], fin_box["bcs"]
                    for h2 in range(2):
                        nc.vector.tensor_mul(
                            aT[hp][h2 * C:(h2 + 1) * C,
                                   qch * QCH:(qch + 1) * QCH],
                            pvs[:, h2 * QCH:(h2 + 1) * QCH],
                            bcs[:, h2 * QCH:(h2 + 1) * QCH])

                # carry closures: the LAG leftover pv pairs + the
                # staged finalize, woven into the NEXT block's slots
                def tail0():
                    pv_pair(TT // 2 - LAG)

                def tail1():
                    pv_pair(TT // 2 - LAG + 1)

                def tail2():
                    pv_pair(TT // 2 - LAG + 2)

                if last:
                    # no on-chip normalize for the final block: ship
                    # raw denominators + unnormalized pvs
                    def last_fin(pv=pv):
                        dga = small.tile([1, 2 * QCH], fp32, tag="dga")
                        pvs = bcsp.tile([C, 2 * QCH], bf16, tag="pvs",
                                        name="pvs")
                        for h2 in range(2):
                            nc.vector.tensor_copy(
                                out=dga[:, h2 * QCH:(h2 + 1) * QCH],
                                in_=pv[h2][C:C + 1, :])
                            _copy(nc.scalar if h2 == 0 else nc.vector,
                                  pvs[:, h2 * QCH:(h2 + 1) * QCH],
                                  pv[h2][0:C, :])
                        nc.scalar.dma_start(out=out3, in_=dga)
                        fin_box["pvs"] = pvs

                    return [tail0, tail1, tail2, last_fin], fin_box
                return [tail0, tail1, tail2, fin_stage1, fin_bc,
                        fin_norm]

            # ---------- emission schedule ----------
            # prologue: kproj(0, ch0) + qproj(0, qch0) halves
            # interleaved so both accumulation chains track the
            # per-dk xt DMA arrivals
            util_state["par"] = 1   # block 0 (par 0) borrows pvB
            bq_p, bk_p = [], []
            qproj_group(0, 0, 0, bq_p)
            kproj_group(0, 0, 0, bk_p)
            qproj_group(0, 0, 1, bq_p)
            kproj_group(0, 0, 1, bk_p)

            carry = None
            blk = 0
            for qch in range(NQCH):
                if qch + 1 < NQCH:
                    load_mask(qch + 1,
                              eng=(nc.sync if qch % 2 == 0 else nc.scalar))
                for hp in range(2):
                    par = blk % 2
                    il = {}

                    def add(pk, f):
                        il.setdefault(pk, []).append(f)

                    # util banks: pk0-2 -> current parity (free until
                    # pv_pair(0) at pk3); pk3+ -> opposite parity
                    # (freed by the carried finalize stage1 at pk3)
                    def set_util(p):
                        def f(p=p):
                            util_state["par"] = p
                        return f

                    add(0, set_util(par))
                    add(3, set_util(1 - par))

                    if qch == 0 and hp == 0:
                        # v tiles (2p, 2p+1) consumed by pv_pair(p) at
                        # pk=p+2 (tails p=6,7 in the next block's
                        # pk0,1); k chunks ch needed by QK at pk=2ch.
                        # NOTE pk2 is a util-bank hole: own-parity
                        # banks start accumulating PV at pk2, opposite
                        # banks are read by the carried finalize until
                        # ~pk3.
                        add(0, lambda: vproj_group(0))
                        add(0, lambda: vproj_group(1))
                        add(0, lambda: kproj_group(0, 1, cp=nc.scalar))
                        add(1, lambda: vproj_group(2))
                        add(1, lambda: vproj_group(3))
                        add(1, lambda: kproj_group(0, 2, cp=nc.scalar))
                        add(2, lambda: vproj_group(4))
                        add(2, lambda: vproj_group(5))
                        add(3, lambda: vproj_group(6))
                        add(3, lambda: vproj_group(7))
                        add(3, lambda: kproj_group(0, 3, cp=nc.scalar))
                        add(4, lambda: vproj_group(8))
                        add(4, lambda: vproj_group(9))
                        add(5, lambda: vproj_group(10))
                        add(5, lambda: vproj_group(11))
                        add(5, lambda: kproj_group(1, 0, cp=nc.scalar))
                        add(6, lambda: vproj_group(12))
                        add(6, lambda: vproj_group(13))
                        bq1 = []
                        add(6, lambda: qproj_group(1, 0, 0, bq1, cp=nc.scalar))
                        add(7, lambda: vproj_group(14))
                        add(7, lambda: vproj_group(15))
                        add(7, lambda: qproj_group(1, 0, 1, bq1, cp=nc.scalar))
                    if qch == 0 and hp == 1:
                        b1, b2, b3 = [], [], []
                        add(0, lambda: kproj_group(1, 1, 0, b1, cp=nc.scalar))
                        add(1, lambda: kproj_group(1, 1, 1, b1, cp=nc.scalar))
                        add(1, lambda: kproj_group(1, 2, 0, b2, cp=nc.scalar))
                        add(2, lambda: kproj_group(1, 2, 1, b2, cp=nc.scalar))
                        add(4, lambda: kproj_group(1, 3, 0, b3, cp=nc.scalar))
                        add(5, lambda: kproj_group(1, 3, 1, b3, cp=nc.scalar))
                    if hp == 0 and qch > 0:
                        # qproj for this qch's hp1 block (needed by its
                        # pk0); opposite banks freed by stage1 ~pk4.2
                        bq = []
                        add(4, lambda q=qch: qproj_group(1, q, 0, bq))
                        add(5, lambda q=qch: qproj_group(1, q, 1, bq))
                        # output projection of the previous qch, in
                        # dch-halves; its aT lands ~pk5.6 (norm at pk5)
                        add(6, lambda q=qch - 1: outproj_half(q, 0, 0))
                        add(7, lambda q=qch - 1: outproj_half(q, 0, 1))
                    if hp == 1 and qch > 0:
                        # pk0-2 use this block's OWN banks (pv starts
                        # at pk3)
                        add(0, lambda q=qch - 1: outproj_half(q, 1, 0))
                        add(1, lambda q=qch - 1: outproj_half(q, 1, 1))
                        add(2, lambda q=qch - 1: outproj_half(q, 2, 0))
                        add(4, lambda q=qch - 1: outproj_half(q, 2, 1))
                        add(6, lambda q=qch - 1: outproj_half(q, 3, 0))
                        add(7, lambda q=qch - 1: outproj_half(q, 3, 1))
                    if hp == 1 and qch + 1 < NQCH:
                        bq0 = []
                        add(4, lambda q=qch: qproj_group(0, q + 1, 0, bq0))
                        add(5, lambda q=qch: qproj_group(0, q + 1, 1, bq0))
                    if hp == 1 and qch == NQCH - 1:
                        # aT[0][q3] lands ~pk5.6 (finalize of (q3,h0)
                        # carried into this block)
                        add(6, lambda: outproj_h0_slab(0))
                        add(7, lambda: outproj_h0_slab(1))
                    last = (qch == NQCH - 1 and hp == 1)
                    r = attn_block(blk, qch, hp, il, carry, last=last)
                    if last:
                        carry, last_box = r
                    else:
                        carry = r
                    blk += 1
            # last block: tails + raw-denominator export, then the
            # unnormalized per-head h1 output rows (no finalize wait)
            for f in carry:
                f()
            util_state["par"] = 0   # last block was par 1
            outproj_h0_slab(2, tail=True)
            outproj_h0_slab(3, tail=True)
            pvs_last = last_box["pvs"]
            for h2 in range(2):
                for tt2 in range(4):
                    outproj_h1_unnorm(pvs_last, h2, tt2)

    nc.compile()
    return nc


def _get_nc():
    if "nc" not in _cache:
        _cache["nc"] = _build()
    return _cache["nc"]


def _make_in_maps(x, mask, Wq, Wk, Wv, Wp):
    x = np.asarray(x, dtype=np.float32)
    mask = np.asarray(mask)
    scale = C ** (-0.5)
    wq_b = (np.asarray(Wq, np.float32) * scale).astype(BF16)
    wk_b = np.asarray(Wk, np.float32).astype(BF16)
    wv_b = np.asarray(Wv, np.float32).astype(BF16)
    wp_b = np.asarray(Wp, np.float32).astype(BF16)

    xTs, maskts = [], []
    for bi in range(B):
        xTs.append(np.ascontiguousarray(x[bi].T).astype(BF16))
        mt = (1 - mask[bi, :, 0, :]).T.astype(np.float32)  # [k, q]
        # -> [qch, kt, 128, 512]
        m4 = mt.reshape(TT, 128, NQCH, QCH).transpose(2, 0, 1, 3)
        maskts.append(np.ascontiguousarray(m4).astype(BF16))

    in_maps = []
    for core in range(NCORES):
        bi, hg = core // HPC, core % HPC
        cr = slice(256 * hg, 256 * (hg + 1))
        in_maps.append({
            "xt": xTs[bi],
            "wq": np.ascontiguousarray(wq_b[:, cr]),
            "wk": np.ascontiguousarray(wk_b[:, cr]),
            "wv": np.ascontiguousarray(wv_b[:, cr]),
            "wp": np.ascontiguousarray(wp_b[cr, :]),
            "maskt": maskts[bi],
        })
    return in_maps


def _run_once(nc, in_maps):
    _import_concourse()
    from concourse.bass_utils import run_bass_kernel_spmd

    res = run_bass_kernel_spmd(nc, in_maps, core_ids=list(range(NCORES)))
    full = np.zeros((B, N, D), np.float32)
    for core in range(NCORES):
        bi = core // HPC
        full[bi] += np.asarray(res.results[core]["out"], np.float32)
        o2 = np.asarray(res.results[core]["out2"], np.float32)
        den = np.asarray(res.results[core]["out3"],
                         np.float32).reshape(2, QCH)
        for h2 in range(2):
            full[bi, (NQCH - 1) * QCH:] += o2[h2] / den[h2][:, None]
    return full


def kernel(x, mask, Wq, Wk, Wv, Wp, bp):
    nc = _get_nc()
    in_maps = _make_in_maps(x, mask, Wq, Wk, Wv, Wp)

    # The device very occasionally returns corrupted results right after a
    # runtime error; run twice and require agreement.
    a = _run_once(nc, in_maps)
    for _ in range(3):
        b = _run_once(nc, in_maps)
        da = np.linalg.norm(a - b) / max(1e-30, np.linalg.norm(b))
        if da < 1e-4:
            break
        a = b
    full = b
    full += np.asarray(bp, np.float32)[None, None, :]
    return full


# revision 28
# speedup vs baseline: 1.0941x; 1.0015x over previous
"""Multi-head attention (AnyAttention) on 8 TRN2 NeuronCores.

Sharding: (batch, head-group): core i handles batch i//4 and heads
4*(i%4) .. 4*(i%4)+4 over ALL 2048 queries (tensor parallel on heads,
row-parallel output projection).  Each core emits a bf16 partial
output [2048, 1024] = attn_out_mine @ Wp[mine_rows]; the host sums
the 4 partials per batch in fp32.

Per-core pipeline (v2 schedule):
  - qT/kT computed c-major [128(=2 heads x 64), 2048]; v token-major
    [128 tok, 4 heads, 65] (65th col = ones -> softmax denominator
    rides the PV matmul stream)
  - logits S^T[k, q]: per (head-pair, qch=512, kt): both heads' QK
    matmuls at PE row bases 0/64 into the two banks of one
    [128, 1024] psum tile; ONE [128, 1024] exp covers both heads
  - mask applied post-exp as mul by host-prepped (1-mask)^T on
    DVE (bf16 2x mode); a few early-slot muls go to GpSimd
  - PV trails QK/exp by LAG=2 slots; the 2 leftover pairs of each
    block are CARRIED into the next block's first two slots so the
    ACT exp cadence never pauses at a block boundary
  - PV psum is double-buffered by block parity (pvA0/pvA1 vs
    pvB0/pvB1, one bank each) so the next block's PV start never
    WAR-waits on the previous block's normalize chain
  - softmax 1/denom: per head, copy the denominator row to
    partition 0, reciprocal_approx_fast [1,512], then
    gpsimd.partition_broadcast into a [64,512] bcs block; the
    normalize multiply reads PV straight out of PSUM.  No PE
    broadcast matmul and no PSUM util bank needed for finalize.
  - proj / outproj / vproj psum lives in the OPPOSITE parity's pv
    banks (free window: after that parity's finalize, before its
    next block), or the CURRENT parity's banks at pk0-1 (free until
    pv_pair(0) at pk2)
  - DMA: critical prefix wq,wk,xt (sync+scalar HW queues) before
    wv, mask0, wp so the first QK can start ~15us in
  - all matmuls bf16 with fp32 PSUM accumulation; scale 1/sqrt(c)
    folded into Wq on host; bp added on host
"""

import numpy as np
import ml_dtypes

B, N, D = 2, 2048, 1024
G, C = 16, 64          # heads, head dim
HPC = 4                # heads per core
NCORES = 8
NQCH = 4               # query chunks of 512
QCH = N // NQCH
TT = N // 128          # 16 token/key tiles
KT = D // 128          # 8 contraction tiles over d

BF16 = ml_dtypes.bfloat16

_cache = {}


def _import_concourse():
    try:
        import concourse.bass  # noqa: F401
    except ImportError:
        import sys
        sys.path.insert(0, "/opt/trn_rl_repo")


def _build():
    _import_concourse()
    import concourse.bass as bass  # noqa: F401
    from concourse import bacc, mybir
    import concourse.tile as tile

    fp32 = mybir.dt.float32
    bf16 = mybir.dt.bfloat16
    AF = mybir.ActivationFunctionType

    nc = bacc.Bacc("TRN2", target_bir_lowering=False, debug=False,
                   num_devices=NCORES)

    # ---- DRAM I/O (per-core shards; same program on all cores) ----
    xt = nc.dram_tensor("xt", [D, N], bf16, kind="ExternalInput").ap()
    wq = nc.dram_tensor("wq", [D, 256], bf16, kind="ExternalInput").ap()
    wk = nc.dram_tensor("wk", [D, 256], bf16, kind="ExternalInput").ap()
    wv = nc.dram_tensor("wv", [D, 256], bf16, kind="ExternalInput").ap()
    wp = nc.dram_tensor("wp", [256, D], bf16, kind="ExternalInput").ap()
    maskt = nc.dram_tensor("maskt", [NQCH, TT, 128, QCH], bf16,
                           kind="ExternalInput").ap()
    out3 = nc.dram_tensor("out3", [1, 2 * QCH], fp32,
                          kind="ExternalOutput").ap()
    out = nc.dram_tensor("out", [N, D], bf16, kind="ExternalOutput").ap()
    # UNNORMALIZED per-head h1 partials of the LAST qch's output
    # rows + their softmax denominators; the host normalizes and
    # adds.  Keeps the end-of-kernel drain off the whole finalize
    # chain (recip -> broadcast -> normalize).
    out2 = nc.dram_tensor("out2", [2, QCH, D], bf16,
                          kind="ExternalOutput").ap()

    GPS_MULS = 0   # h2=1 mask muls of the LAST GPS_MULS pks go to GpSimd
    # (GpSimd muls run concurrently with DVE muls on the same mask
    # slice and SBUF contention slows the DVE mul 0.6us -> 2.5us;
    # all-DVE is faster overall)

    with tile.TileContext(nc) as tc:
        with (
            tc.tile_pool(name="wts", bufs=3) as wpool,
            tc.tile_pool(name="wpp", bufs=1) as wppool,
            tc.tile_pool(name="xtp", bufs=1) as xtpool,
            tc.tile_pool(name="maskp", bufs=2) as maskpool,
            tc.tile_pool(name="stay", bufs=1) as stay,
            tc.tile_pool(name="etp", bufs=2) as etpool,
            tc.tile_pool(name="metp", bufs=3) as metpool,
            tc.tile_pool(name="bcsp", bufs=2) as bcsp,
            tc.tile_pool(name="small", bufs=1) as small,
            tc.tile_pool(name="outp", bufs=2) as outpool,
            tc.tile_pool(name="psqk", bufs=2, space="PSUM") as psqk,
            tc.tile_pool(name="pspv", bufs=1, space="PSUM") as pspv,
        ):
            # ---------- DMA: critical prefix first ----------
            # sync HW queue: wq, wk, xt evens; scalar HW queue: xt
            # odds, wv, mask0, wp.  The first QK needs wq+wk+xt; wv
            # by ~pk0 of block 0; mask0 by the first mask mul; wp
            # not until the first outproj (block 2).
            w_t = {}
            for name, dr in (("wq", wq), ("wk", wk)):
                wt = wpool.tile([128, KT * 256], bf16, tag=name, name=name)
                nc.sync.dma_start(
                    out=wt.rearrange("p (dk c) -> p dk c", dk=KT),
                    in_=dr.rearrange("(dk p) c -> p dk c", p=128))
                w_t[name] = [wt[:, dk * 256:(dk + 1) * 256]
                             for dk in range(KT)]
            xt_t = []
            for dk in range(KT):
                t = xtpool.tile([128, N], bf16, tag=f"xt{dk}")
                eng = nc.sync if dk % 2 == 0 else nc.scalar
                eng.dma_start(out=t, in_=xt[dk * 128:(dk + 1) * 128, :])
                xt_t.append(t)
            for name, dr in (("wv", wv),):
                wt = wpool.tile([128, KT * 256], bf16, tag=name, name=name)
                nc.scalar.dma_start(
                    out=wt.rearrange("p (dk c) -> p dk c", dk=KT),
                    in_=dr.rearrange("(dk p) c -> p dk c", p=128))
                w_t[name] = [wt[:, dk * 256:(dk + 1) * 256]
                             for dk in range(KT)]

            mask_t = {}

            def load_mask(qch, eng=None):
                mt = maskpool.tile([128, TT * QCH], bf16, tag="mask")
                (eng or nc.sync).dma_start(
                    out=mt.rearrange("p (kt q) -> p kt q", kt=TT),
                    in_=maskt[qch].rearrange("kt p q -> p kt q"))
                mask_t[qch] = mt

            load_mask(0, eng=nc.scalar)
            wpt = wppool.tile([128, 2 * D], bf16, tag="wp", name="wp")
            nc.scalar.dma_start(
                out=wpt.rearrange("p (r c) -> p r c", r=2),
                in_=wp.rearrange("(r p) c -> p r c", p=128))
            w_t["wp"] = [wpt[:, r * D:(r + 1) * D] for r in range(2)]
            # head-1 half of Wp's hp1 rows re-based to partition 0 for
            # the per-head unnormalized tail matmuls
            wp1b = stay.tile([C, D], bf16, tag="wp1b", name="wp1b")

            # touch Exp now so ACT_TABLE_LOAD is off the critical path
            warm = small.tile([1, C], bf16, tag="warm")
            nc.vector.memset(warm, 1.0)
            nc.scalar.activation(out=warm, in_=warm, func=AF.Exp)
            nc.vector.tensor_copy(out=wp1b,
                                  in_=wpt[C:2 * C, D:2 * D])

            qT = [stay.tile([128, N], bf16, tag=f"qT{hp}", name=f"qT{hp}")
                  for hp in range(2)]
            kTt = [stay.tile([128, N], bf16, tag=f"kT{hp}", name=f"kT{hp}")
                   for hp in range(2)]
            aT = [stay.tile([128, N], bf16, tag=f"aT{hp}", name=f"aT{hp}")
                  for hp in range(2)]
            v_t = [stay.tile([128, HPC, C + 1], bf16, tag=f"v{tt}",
                             name=f"v{tt}")
                   for tt in range(TT)]

            # ---------- PSUM: 4 banks QK + 4 banks PV (parity) ----------
            # pv tiles double as proj/outproj/vproj scratch ("util")
            # during their idle parity windows.
            def pv_tile(par, h2):
                return pspv.tile([128, QCH], fp32, tag=f"pv{par}{h2}",
                                 name=f"pv{par}{h2}")

            util_state = {"par": 0, "h2": 0}

            def util_tile():
                # rotate over the two banks of the currently-safe parity
                h2 = util_state["h2"]
                util_state["h2"] = 1 - h2
                return pv_tile(util_state["par"], h2)

            # ---------- worker emitters ----------
            def _copy(eng, out, in_):
                # ACT 'copy' absorbs PSUM->SBUF casts on slots where
                # the exp pipeline is idle anyway (blocks 0-1, tail)
                if eng is nc.scalar:
                    nc.scalar.copy(out=out, in_=in_)
                else:
                    nc.vector.tensor_copy(out=out, in_=in_)

            def _proj_half(wname, dst, hp, ch, part, box, cp=None):
                if part == 0:
                    box.clear()
                    box.append(util_tile())
                ps = box[0]
                for dk in range(part * 4, part * 4 + 4):
                    nc.tensor.matmul(
                        ps, w_t[wname][dk][:, hp * 128:(hp + 1) * 128],
                        xt_t[dk][:, ch * QCH:(ch + 1) * QCH],
                        start=(dk == 0), stop=(dk == KT - 1))
                if part == 1:
                    _copy(cp or nc.vector,
                          dst[hp][:, ch * QCH:(ch + 1) * QCH], ps)

            def qproj_group(hp, qch, part=None, box=[], cp=None):
                if part is None:
                    _proj_half("wq", qT, hp, qch, 0, box, cp)
                    _proj_half("wq", qT, hp, qch, 1, box, cp)
                else:
                    _proj_half("wq", qT, hp, qch, part, box, cp)

            def kproj_group(hp, ch, part=None, box=[], cp=None):
                if part is None:
                    _proj_half("wk", kTt, hp, ch, 0, box, cp)
                    _proj_half("wk", kTt, hp, ch, 1, box, cp)
                else:
                    _proj_half("wk", kTt, hp, ch, part, box, cp)

            def vproj_group(tt):
                ps = util_tile()
                for dk in range(KT):
                    nc.tensor.matmul(
                        ps[:, 0:256],
                        xt_t[dk][:, tt * 128:(tt + 1) * 128],
                        w_t["wv"][dk],
                        start=(dk == 0), stop=(dk == KT - 1))
                vt = v_t[tt]
                nc.vector.memset(vt[:, :, C:C + 1], 1.0)
                nc.scalar.copy(
                    out=vt[:, :, 0:C],
                    in_=ps[:, 0:256].rearrange("p (h c) -> p h c", c=C))

            def outproj_h0_slab(tt2, tail=False):
                # normal h0 output rows for the LAST qch
                r0 = (NQCH - 1) * QCH + tt2 * 128
                osb = outpool.tile([128, D], bf16, tag="osb")
                for dch in range(2):
                    ps = util_tile()
                    nc.tensor.matmul(
                        ps, aT[0][:, r0:r0 + 128],
                        w_t["wp"][0][:, dch * QCH:(dch + 1) * QCH],
                        start=True, stop=True)
                    _copy(nc.scalar if (tail and dch == 0) else nc.vector,
                          osb[:, dch * QCH:(dch + 1) * QCH], ps)
                nc.sync.dma_start(out=out[r0:r0 + 128, :], in_=osb)

            def outproj_h1_unnorm(pvs, h2, tt2):
                # per-head UNNORMALIZED h1 output rows of the last
                # qch, straight from the pvs cast (no finalize wait);
                # casts split between DVE and the now-idle ACT
                osb = outpool.tile([128, D], bf16, tag="osb")
                for dch in range(2):
                    ps = util_tile()
                    if h2 == 0:
                        rhs = w_t["wp"][1][0:C,
                                           dch * QCH:(dch + 1) * QCH]
                    else:
                        rhs = wp1b[:, dch * QCH:(dch + 1) * QCH]
                    nc.tensor.matmul(
                        ps[:, :],
                        pvs[:, h2 * QCH + tt2 * 128:
                            h2 * QCH + (tt2 + 1) * 128],
                        rhs, start=True, stop=True)
                    _copy(nc.scalar if dch == 0 else nc.vector,
                          osb[:, dch * QCH:(dch + 1) * QCH], ps)
                eng = nc.sync if (h2 * 4 + tt2) % 2 == 0 else nc.scalar
                eng.dma_start(
                    out=out2[h2, tt2 * 128:(tt2 + 1) * 128, :], in_=osb)

            osb_box = {}

            def outproj_half(qch, tt2, dch):
                # one dch half of one 128-token output row block; the
                # DMA fires with the second half
                if dch == 0:
                    osb_box[tt2] = outpool.tile([128, D], bf16,
                                                tag="osb", name="osb")
                osb = osb_box[tt2]
                r0 = qch * QCH + tt2 * 128
                ps = util_tile()
                for hp in range(2):
                    nc.tensor.matmul(
                        ps, aT[hp][:, r0:r0 + 128],
                        w_t["wp"][hp][:, dch * QCH:(dch + 1) * QCH],
                        start=(hp == 0), stop=(hp == 1))
                nc.vector.tensor_copy(
                    out=osb[:, dch * QCH:(dch + 1) * QCH], in_=ps)
                if dch == 1:
                    nc.sync.dma_start(out=out[r0:r0 + 128, :], in_=osb)

            # ---------- attention block for (qch, hp) ----------
            # PV trails the unit mask-mul by a full slot (the mul for
            # unit u lands end of pk 2u+1; pv_pair(2u) runs at 2u+3)
            LAG = 3

            def attn_block(blk, qch, hp, interleave, carry_in,
                           last=False):
                par = blk % 2
                mt = mask_t[qch]
                pv = [pv_tile(par, h2)[0:C + 1, :] for h2 in range(2)]
                # et/met tiles cover a 2-pair UNIT (4 kt): the mask
                # multiply is then ONE out-of-place [128,2048] bf16
                # tensor_tensor per head per unit, which the DVE runs
                # in 4x mode (~690ns vs 2x 601ns per [128,1024])
                et_units = [None] * (TT // 4)
                met_units = [None] * (TT // 4)

                def pv_pair(pk2, pv=pv, hp=hp):
                    metu = met_units[pk2 // 2]
                    pr = pk2 % 2
                    for j in range(2):
                        kt = 2 * pk2 + j
                        for h2 in range(2):
                            nc.tensor.matmul(
                                pv[h2], v_t[kt][:, hp * 2 + h2, :],
                                metu[:, h2, pr, j, :],
                                start=(kt == 0), stop=(kt == TT - 1))

                for pk in range(TT // 2):
                    u, pr = pk // 2, pk % 2
                    if pr == 0:
                        et_units[u] = etpool.tile(
                            [128, 2, 2, 2, QCH], bf16, tag="et",
                            name="et")
                        met_units[u] = metpool.tile(
                            [128, 2, 2, 2, QCH], bf16, tag="met",
                            name="met")
                    et = et_units[u]
                    for j in ((0, 1) if pk % 2 == 0 else (1, 0)):
                        kt = 2 * pk + j
                        ps = psqk.tile([128, 2 * QCH], fp32, tag="qk")
                        for h2 in range(2):
                            pb = h2 * C
                            nc.tensor.matmul(
                                ps[:, h2 * QCH:(h2 + 1) * QCH],
                                kTt[hp][pb:pb + C,
                                        kt * 128:(kt + 1) * 128],
                                qT[hp][pb:pb + C,
                                       qch * QCH:(qch + 1) * QCH],
                                start=True, stop=True)
                        nc.scalar.activation(
                            out=et[:, :, pr, j, :],
                            in_=ps.rearrange("p (h q) -> p h q", h=2),
                            func=AF.Exp)
                    # carry first: its PE ops (tail pv pairs, pk0-1)
                    # depend only on the previous block's tiles, and
                    # its DVE ops (finalize stage 1, pk2) must beat
                    # this slot's mask muls into the DVE FIFO so the
                    # pv banks free on time
                    if carry_in is not None and pk < len(carry_in):
                        carry_in[pk]()
                    if pr == 1:
                        met = met_units[u]
                        msl = mt[:, u * 4 * QCH:(u + 1) * 4 * QCH]
                        for h2 in range(2):
                            nc.vector.tensor_mul(
                                met[:, h2].rearrange(
                                    "p a b c -> p (a b c)"),
                                et[:, h2].rearrange(
                                    "p a b c -> p (a b c)"),
                                msl)
                    if pk >= LAG:
                        pv_pair(pk - LAG)
                    for f in interleave.get(pk, ()):
                        f()

                # Finalize, staged across carry slots so no DVE op
                # ever waits in-FIFO on the gpsimd broadcast:
                #   pk2: denominator rows -> [1,1024] row, pv value
                #        rows -> SBUF bf16 (frees the pv psum banks)
                #   pk3: recip [1,1024] + bf16 cast + gpsimd
                #        broadcast to [64,1024]
                #   pk5: 2x-mode bf16 normalize multiplies into aT
                fin_box = {}

                def fin_stage1(pv=pv):
                    # recip chain first (it gates the bc matmuls at
                    # pk3), then the pv value casts (they gate only
                    # the pk4+ norm muls / bank release)
                    dga = small.tile([1, 2 * QCH], fp32, tag="dga")
                    for h2 in range(2):
                        nc.vector.tensor_copy(
                            out=dga[:, h2 * QCH:(h2 + 1) * QCH],
                            in_=pv[h2][C:C + 1, :])
                    rcf = small.tile([1, 2 * QCH], fp32, tag="rcf")
                    nc.vector.reciprocal_approx_fast(rcf, dga)
                    rcb = small.tile([1, 2 * QCH], bf16, tag="rcb")
                    nc.vector.tensor_copy(out=rcb, in_=rcf)
                    fin_box["rcb"] = rcb
                    pvs = bcsp.tile([C, 2 * QCH], bf16, tag="pvs",
                                    name="pvs")
                    for h2 in range(2):
                        nc.vector.tensor_copy(
                            out=pvs[:, h2 * QCH:(h2 + 1) * QCH],
                            in_=pv[h2][0:C, :])
                    fin_box["pvs"] = pvs

                def fin_bc():
                    # broadcast the recip row to 64 partitions on the
                    # (otherwise idle) GpSimd -- its library never
                    # switches, so no reload cost
                    bcs = bcsp.tile([C, 2 * QCH], bf16, tag="bcs",
                                    name="bcs")
                    nc.gpsimd.partition_broadcast(bcs, fin_box["rcb"])
                    fin_box["bcs"] = bcs

                def fin_norm(hp=hp, qch=qch):
                    pvs, bcs = fin_box["pvs"], fin_box["bcs"]
                    for h2 in range(2):
                        nc.vector.tensor_mul(
                            aT[hp][h2 * C:(h2 + 1) * C,
                                   qch * QCH:(qch + 1) * QCH],
                            pvs[:, h2 * QCH:(h2 + 1) * QCH],
                            bcs[:, h2 * QCH:(h2 + 1) * QCH])

                # carry closures: the LAG leftover pv pairs + the
                # staged finalize, woven into the NEXT block's slots
                def tail0():
                    pv_pair(TT // 2 - LAG)

                def tail1():
                    pv_pair(TT // 2 - LAG + 1)

                def tail2():
                    pv_pair(TT // 2 - LAG + 2)

                if last:
                    # no on-chip normalize for the final block: ship
                    # raw denominators + unnormalized pvs
                    def last_fin(pv=pv):
                        dga = small.tile([1, 2 * QCH], fp32, tag="dga")
                        pvs = bcsp.tile([C, 2 * QCH], bf16, tag="pvs",
                                        name="pvs")
                        for h2 in range(2):
                            nc.vector.tensor_copy(
                                out=dga[:, h2 * QCH:(h2 + 1) * QCH],
                                in_=pv[h2][C:C + 1, :])
                            _copy(nc.scalar if h2 == 0 else nc.vector,
                                  pvs[:, h2 * QCH:(h2 + 1) * QCH],
                                  pv[h2][0:C, :])
                        nc.scalar.dma_start(out=out3, in_=dga)
                        fin_box["pvs"] = pvs

                    return [tail0, tail1, tail2, last_fin], fin_box
                return [tail0, tail1, tail2, fin_stage1, fin_bc,
                        fin_norm]

            # ---------- emission schedule ----------
            # prologue: kproj(0, ch0) + qproj(0, qch0) halves
            # interleaved so both accumulation chains track the
            # per-dk xt DMA arrivals
            util_state["par"] = 1   # block 0 (par 0) borrows pvB
            bq_p, bk_p = [], []
            qproj_group(0, 0, 0, bq_p)
            kproj_group(0, 0, 0, bk_p)
            qproj_group(0, 0, 1, bq_p)
            kproj_group(0, 0, 1, bk_p)

            carry = None
            blk = 0
            for qch in range(NQCH):
                if qch + 1 < NQCH:
                    load_mask(qch + 1,
                              eng=(nc.sync if qch % 2 == 0 else nc.scalar))
                for hp in range(2):
                    par = blk % 2
                    il = {}

                    def add(pk, f):
                        il.setdefault(pk, []).append(f)

                    # util banks: pk0-2 -> current parity (free until
                    # pv_pair(0) at pk3); pk3+ -> opposite parity
                    # (freed by the carried finalize stage1 at pk3)
                    def set_util(p):
                        def f(p=p):
                            util_state["par"] = p
                        return f

                    add(0, set_util(par))
                    add(3, set_util(1 - par))

                    if qch == 0 and hp == 0:
                        # v tiles (2p, 2p+1) consumed by pv_pair(p) at
                        # pk=p+2 (tails p=6,7 in the next block's
                        # pk0,1); k chunks ch needed by QK at pk=2ch.
                        # NOTE pk2 is a util-bank hole: own-parity
                        # banks start accumulating PV at pk2, opposite
                        # banks are read by the carried finalize until
                        # ~pk3.
                        add(0, lambda: vproj_group(0))
                        add(0, lambda: vproj_group(1))
                        add(0, lambda: kproj_group(0, 1, cp=nc.scalar))
                        add(1, lambda: vproj_group(2))
                        add(1, lambda: vproj_group(3))
                        add(1, lambda: kproj_group(0, 2, cp=nc.scalar))
                        add(2, lambda: vproj_group(4))
                        add(2, lambda: vproj_group(5))
                        add(3, lambda: vproj_group(6))
                        add(3, lambda: vproj_group(7))
                        add(3, lambda: kproj_group(0, 3, cp=nc.scalar))
                        add(4, lambda: vproj_group(8))
                        add(4, lambda: vproj_group(9))
                        add(5, lambda: vproj_group(10))
                        add(5, lambda: vproj_group(11))
                        add(5, lambda: kproj_group(1, 0, cp=nc.scalar))
                        add(6, lambda: vproj_group(12))
                        add(6, lambda: vproj_group(13))
                        bq1 = []
                        add(6, lambda: qproj_group(1, 0, 0, bq1, cp=nc.scalar))
                        add(7, lambda: vproj_group(14))
                        add(7, lambda: vproj_group(15))
                        add(7, lambda: qproj_group(1, 0, 1, bq1, cp=nc.scalar))
                    if qch == 0 and hp == 1:
                        b1, b2, b3 = [], [], []
                        add(0, lambda: kproj_group(1, 1, 0, b1, cp=nc.scalar))
                        add(1, lambda: kproj_group(1, 1, 1, b1, cp=nc.scalar))
                        add(1, lambda: kproj_group(1, 2, 0, b2, cp=nc.scalar))
                        add(2, lambda: kproj_group(1, 2, 1, b2, cp=nc.scalar))
                        add(4, lambda: kproj_group(1, 3, 0, b3, cp=nc.scalar))
                        add(5, lambda: kproj_group(1, 3, 1, b3, cp=nc.scalar))
                    if hp == 0 and qch > 0:
                        # qproj for this qch's hp1 block (needed by its
                        # pk0); opposite banks freed by stage1 ~pk4.2
                        bq = []
                        add(4, lambda q=qch: qproj_group(1, q, 0, bq))
                        add(5, lambda q=qch: qproj_group(1, q, 1, bq))
                        # output projection of the previous qch, in
                        # dch-halves; its aT lands ~pk5.6 (norm at pk5)
                        add(6, lambda q=qch - 1: outproj_half(q, 0, 0))
                        add(7, lambda q=qch - 1: outproj_half(q, 0, 1))
                    if hp == 1 and qch > 0:
                        # pk0-2 use this block's OWN banks (pv starts
                        # at pk3)
                        add(0, lambda q=qch - 1: outproj_half(q, 1, 0))
                        add(1, lambda q=qch - 1: outproj_half(q, 1, 1))
                        add(2, lambda q=qch - 1: outproj_half(q, 2, 0))
                        add(4, lambda q=qch - 1: outproj_half(q, 2, 1))
                        add(6, lambda q=qch - 1: outproj_half(q, 3, 0))
                        add(7, lambda q=qch - 1: outproj_half(q, 3, 1))
                    if hp == 1 and qch + 1 < NQCH:
                        bq0 = []
                        add(4, lambda q=qch: qproj_group(0, q + 1, 0, bq0))
                        add(5, lambda q=qch: qproj_group(0, q + 1, 1, bq0))
                    if hp == 1 and qch == NQCH - 1:
                        # aT[0][q3] lands ~pk5.6 (finalize of (q3,h0)
                        # carried into this block)
                        add(6, lambda: outproj_h0_slab(0))
                        add(7, lambda: outproj_h0_slab(1))
                    last = (qch == NQCH - 1 and hp == 1)
                    r = attn_block(blk, qch, hp, il, carry, last=last)
                    if last:
                        carry, last_box = r
                    else:
                        carry = r
                    blk += 1
            # last block: tails + raw-denominator export, then the
            # unnormalized per-head h1 output rows (no finalize wait)
            for f in carry:
                f()
            # tail: rotate utils over ALL FOUR pv banks (both
            # parities are dead) so the slab matmuls never wait on a
            # cast; casts split across DVE+ACT
            util_state["par"] = 0   # last block was par 1
            outproj_h0_slab(2, tail=True)
            util_state["par"] = 1
            outproj_h0_slab(3, tail=True)
            pvs_last = last_box["pvs"]
            for i, (h2, tt2) in enumerate(
                    (h, t) for h in range(2) for t in range(4)):
                util_state["par"] = i % 2
                outproj_h1_unnorm(pvs_last, h2, tt2)

    nc.compile()
    return nc


def _get_nc():
    if "nc" not in _cache:
        _cache["nc"] = _build()
    return _cache["nc"]


def _make_in_maps(x, mask, Wq, Wk, Wv, Wp):
    x = np.asarray(x, dtype=np.float32)
    mask = np.asarray(mask)
    scale = C ** (-0.5)
    wq_b = (np.asarray(Wq, np.float32) * scale).astype(BF16)
    wk_b = np.asarray(Wk, np.float32).astype(BF16)
    wv_b = np.asarray(Wv, np.float32).astype(BF16)
    wp_b = np.asarray(Wp, np.float32).astype(BF16)

    xTs, maskts = [], []
    for bi in range(B):
        xTs.append(np.ascontiguousarray(x[bi].T).astype(BF16))
        mt = (1 - mask[bi, :, 0, :]).T.astype(np.float32)  # [k, q]
        # -> [qch, kt, 128, 512]
        m4 = mt.reshape(TT, 128, NQCH, QCH).transpose(2, 0, 1, 3)
        maskts.append(np.ascontiguousarray(m4).astype(BF16))

    in_maps = []
    for core in range(NCORES):
        bi, hg = core // HPC, core % HPC
        cr = slice(256 * hg, 256 * (hg + 1))
        in_maps.append({
            "xt": xTs[bi],
            "wq": np.ascontiguousarray(wq_b[:, cr]),
            "wk": np.ascontiguousarray(wk_b[:, cr]),
            "wv": np.ascontiguousarray(wv_b[:, cr]),
            "wp": np.ascontiguousarray(wp_b[cr, :]),
            "maskt": maskts[bi],
        })
    return in_maps


def _run_once(nc, in_maps):
    _import_concourse()
    from concourse.bass_utils import run_bass_kernel_spmd

    res = run_bass_kernel_spmd(nc, in_maps, core_ids=list(range(NCORES)))
    full = np.zeros((B, N, D), np.float32)
    for core in range(NCORES):
        bi = core // HPC
        full[bi] += np.asarray(res.results[core]["out"], np.float32)
        o2 = np.asarray(res.results[core]["out2"], np.float32)
        den = np.asarray(res.results[core]["out3"],
                         np.float32).reshape(2, QCH)
        for h2 in range(2):
            full[bi, (NQCH - 1) * QCH:] += o2[h2] / den[h2][:, None]
    return full


def kernel(x, mask, Wq, Wk, Wv, Wp, bp):
    nc = _get_nc()
    in_maps = _make_in_maps(x, mask, Wq, Wk, Wv, Wp)

    # The device very occasionally returns corrupted results right after a
    # runtime error; run twice and require agreement.
    a = _run_once(nc, in_maps)
    for _ in range(3):
        b = _run_once(nc, in_maps)
        da = np.linalg.norm(a - b) / max(1e-30, np.linalg.norm(b))
        if da < 1e-4:
            break
        a = b
    full = b
    full += np.asarray(bp, np.float32)[None, None, :]
    return full
